# revision 1
# baseline (speedup 1.0000x reference)
"""Trainium2 Bass kernel for nn_EventPixelFF.

Pipeline (per NeuronCore, 8-way sharded over W columns):
  - host: shard events by output column (with conv halo), sort by output
    cell, bucket into 480-cell slabs; compute gather indices into a
    per-core pre-expanded hash table (8 corners baked per cell, feature-
    major); repack conv/MLP weights into lhsT layout.
  - device: trilinear weights on DVE, table rows via gpsimd dma_gather,
    weighted corner-reduce on DVE, scatter into the feature grid via
    one-hot matmuls on PE (PSUM-accumulated per slab), then the conv
    stack + per-pixel MLP as fp32r matmuls with shifted-view rhs.
"""

import sys
import numpy as np

for _p in ("/opt/trn_rl_repo", "/root/.axon_site/_ro/trn_rl_repo"):
    if _p not in sys.path:
        sys.path.insert(0, _p)

# ---------------------------------------------------------------- constants
L = 8
F = 4
TBL = 1 << 19
W, H = 320, 180
T = 20
B = 2
C0 = 128
NCORES = 8
WS = 40          # output W columns per core
HALO = 4
WL = WS + 2 * HALO   # 48 local columns with halo

# resolutions exactly as reference._resolutions() computes them (float64
# log/exp path, margins ~1e-6 -- safe to hardcode)
RES = np.array([[16, 16, 4], [24, 22, 5], [37, 31, 8], [57, 45, 11],
                [88, 63, 16], [135, 90, 24], [208, 127, 34], [320, 179, 50]],
               dtype=np.int32)
PRIMES = np.array([1, 2654435761, 805459861], dtype=np.uint32)

NY = [int(RES[l][1]) + 1 for l in range(L)]
NT = [int(RES[l][2]) + 1 for l in range(L)]

# per-core local table x-extent. For core with first local column at global
# cx0: events at local column lw have global ix = floor(x*rx) with
# x*320 in [cx0+lw-0.5, cx0+lw+0.5] =>
#   ix in [floor((2*(cx0+lw)-1)*rx/640), floor((2*(cx0+lw)+1)*rx/640)]
# (exact integer arithmetic; +-1 f32 safety added except rx=320 where the
# relation is exact because the same f32 product feeds round and floor).

def _ix_min(cxg, l):
    rx = int(RES[l][0])
    v = ((2 * cxg - 1) * rx) // 640
    return v if rx == 320 else v - 1


def _ix_max(cxg, l):
    rx = int(RES[l][0])
    v = ((2 * cxg + 1) * rx) // 640
    return v if rx == 320 else v + 1


def _ixbase(core, l):
    cx0 = WS * core - HALO
    return _ix_min(cx0, l)


IXW = []
for l in range(L):
    w = 0
    for core in range(NCORES):
        cx0 = WS * core - HALO
        w = max(w, _ix_max(cx0 + WL - 1, l) + 1 - _ixbase(core, l) + 1)
    IXW.append(w)
LBASE = np.concatenate([[0], np.cumsum([IXW[l] * NY[l] * NT[l] for l in range(L)])]).astype(np.int64)
TOTROW = int(LBASE[-1])
EST = 64  # elements per expanded-table row (32 used + pad to 256B)

# token geometry: 240-cell half-buckets; image-edge columns (cx=0/319, i.e.
# local lw=4 on core 0 and lw=47 on core 7) catch 1.5x density from clipping,
# so buckets 3 and 35 get extra capacity.
HB = 240
NHB = (WL * H) // HB            # 36 half-buckets per frame
PAD_CELLREL = 10000.0
BANDR = 32768  # int16 index reach
SLABMAX = HB      # one-hot width (240)

# slab entries: one 240-cell bucket each (nb=2 slabs fail HW execution
# on this runtime -- redacted INTERNAL error; keep nb=1)
SLABS = [(k, 1) for k in range(NHB)]

# gather calls per slab entry: levels 0..6 whole entry; level 7 per bucket
def _entry_calls(k0, nb):
    calls = [(l, -1) for l in range(7)]
    for hb in range(nb):
        calls.append((7, hb) if nb > 1 else (7, -1))
    return calls


def _set_caps(tcap):
    """(Re)derive the token geometry from per-bucket tile capacities."""
    global TCAP, HBOFF, TILESF, TOKF, NTOK, TPSMAX, IDXCOLS_ALL
    TCAP = list(tcap)
    HBOFF = np.concatenate([[0], np.cumsum(TCAP)]).astype(np.int64)
    TILESF = int(HBOFF[-1])
    TOKF = TILESF * 128
    NTOK = B * TOKF
    TPSMAX = max(sum(TCAP[k0:k0 + nb]) for (k0, nb) in SLABS)
    IDXCOLS_ALL = B * sum(
        (sum(TCAP[k0:k0 + nb]) * 8) * 7 + sum(TCAP[k0 + hb] * 8 for hb in range(nb))
        for (k0, nb) in SLABS)


_set_caps([8 if k in (3, 35) else 6 for k in range(NHB)])

# conv strip geometry. Strip s computes output rows cy in [45s, 45s+45)
# at window t = cy - cy0 with cy0 = 45s - 4. Each conv layer shrinks the
# computed window by one row per side (validity cascade); windows are
# clamped to the image so out-of-image rows stay memset-zero (= conv
# 'SAME' padding), and lengths forced even (fp32r matmul ISA rule). When
# parity needs fixing: if the bottom is image-clamped, shrink the top
# (the lost top row only feeds outputs that are discarded); else extend
# down one junk row that no later layer reads.
NS = 4
SROWS = H // NS                  # 45 output rows per strip
TT = SROWS + 8                   # 53 rows per strip window
CY0 = [SROWS * s - 4 for s in range(NS)]
MLO = 4                          # output rows are t in [4, 49)


def _strip_windows(cy0):
    lo1 = max(1, -cy0)
    hi1 = min(TT, H - cy0)
    if (hi1 - lo1) % 2:
        if -cy0 >= 1:
            hi1 -= 1
        else:
            lo1 -= 1
    ws = {1: (lo1, hi1)}
    for k, (bl, bh) in ((2, (2, 52)), (3, (3, 51)), (4, (4, 50))):
        lo = max(bl, -cy0)
        hi = min(bh, H - cy0)
        if (hi - lo) % 2:
            if -cy0 >= bl:
                hi -= 1
            else:
                lo -= 1
        ws[k] = (lo, hi)
    assert ws[4][1] - ws[4][0] == SROWS + 1, (cy0, ws)
    return ws


NLW = 50                         # lw blocks incl +-1 pads (real lw at block lw+1)
LWQ = 17                         # ceil(50/3) : x0 packed [f+32*(lwb%3), (lwb//3)*182+u]
X0C = LWQ * (H + 2) + 2          # 3094 cols + 2 pad
CPSW = 6 * 52                    # conv psum tile width (max 6 lws x 52 rows)

OUTPX = B * WS * H               # 14400


def _band_from_lws(l, lwlo, lwhi):
    """core-independent band start: min over cores of the local ix lower
    bound for columns [lwlo, lwhi]; asserts the worst-case width fits."""
    lo = min(_ix_min(WS * core - HALO + lwlo, l) - _ixbase(core, l)
             for core in range(NCORES))
    hi = max(_ix_max(WS * core - HALO + lwhi, l) - _ixbase(core, l)
             for core in range(NCORES))
    lo = max(0, lo)
    assert (hi - lo + 1) * NY[l] * NT[l] <= BANDR, (l, lwlo, lwhi, lo, hi)
    start = lo * NY[l] * NT[l]
    maxstart = max(0, IXW[l] * NY[l] * NT[l] - BANDR)
    return min(start, maxstart)


def _cells_band(l, c0, c1):
    return _band_from_lws(l, c0 // H, c1 // H)


# ---------------------------------------------------------------- host prep

def _expand_tables(hash_tables):
    """Build the per-core-family expanded tables. Returns list of 8 arrays
    [TOTROW, EST] float32 (one per core; they differ by x-slice)."""
    ht = np.asarray(hash_tables, dtype=np.float32)  # (L, TBL, F)
    out = []
    for core in range(NCORES):
        exp = np.zeros((TOTROW, EST), dtype=np.float32)
        cx0 = WS * core - HALO  # global cx of local lw=0
        for l in range(L):
            rx = int(RES[l][0])
            ny, nt = NY[l], NT[l]
            ixbase = _ixbase(core, l)
            ixs = ixbase + np.arange(IXW[l])              # global ix values
            iys = np.arange(ny)
            its = np.arange(nt)
            IX, IY, IT = np.meshgrid(ixs, iys, its, indexing="ij")
            rows = np.zeros((IXW[l], ny, nt, F, 8), dtype=np.float32)
            for c in range(8):
                ox, oy, ot = c & 1, (c >> 1) & 1, (c >> 2) & 1
                a = (IX + ox).astype(np.int64)
                # clip negatives/overflow: events never index there, value irrelevant
                a = np.clip(a, 0, None).astype(np.uint32)
                b = (IY + oy).astype(np.uint32)
                d = (IT + ot).astype(np.uint32)
                hsh = ((a * PRIMES[0]) ^ (b * PRIMES[1]) ^ (d * PRIMES[2])) % TBL
                rows[:, :, :, :, c] = ht[l][hsh.astype(np.int64)]
            exp[LBASE[l]:LBASE[l + 1], :32] = rows.reshape(-1, 32)
        out.append(exp)
    return out


def _prep_events(currentBlock, eventCounts):
    """Shard + sort + slab-bucket events. Returns per-core dicts with
    coords [128, 4*NTOK/128] f32 and idx16 [128, IDXCOLS_ALL] i16,
    or None if any slab overflows its fixed capacity."""
    cb = np.asarray(currentBlock, dtype=np.float32)
    ec = np.asarray(eventCounts).astype(np.int64)
    n = cb.shape[0]
    bidx = np.repeat(np.arange(B, dtype=np.int64), ec, )
    if bidx.shape[0] != n:
        # pad/truncate defensively (counts should sum to N)
        bidx = np.resize(bidx, n)
    x, y = cb[:, 0], cb[:, 1]
    cx = np.clip(np.round(x * np.float32(W)), 0, W - 1).astype(np.int64)
    cy = np.clip(np.round(y * np.float32(H)), 0, H - 1).astype(np.int64)

    # per (event, level) local cell ids (host-side integer planning)
    scaled = cb[:, None, :] * RES[None, :, :].astype(np.float32)   # (n, L, 3)
    base = np.floor(scaled).astype(np.int64)                        # (n, L, 3)

    cores = []
    for core in range(NCORES):
        cx0 = WS * core - HALO
        sel = (cx >= cx0) & (cx < cx0 + WL)
        ei = np.nonzero(sel)[0]
        lw = cx[ei] - cx0
        cell = lw * H + cy[ei]
        fr = bidx[ei]
        order = np.lexsort((cell, fr))
        ei, lw, cell, fr = ei[order], lw[order], cell[order], fr[order]

        coords = np.full((128, (NTOK // 128) * 4), 0.5, dtype=np.float32)
        coords[:, 3::4] = PAD_CELLREL
        idx16 = np.zeros((16, IDXCOLS_ALL), dtype=np.int16)

        ok = True
        for f in range(B):
            m = fr == f
            ce = cell[m]
            eidx = ei[m]
            hb_of = ce // HB
            cc = f * (IDXCOLS_ALL // B)
            for (k0, nb) in SLABS:
                # place events of each bucket into its fixed tile range
                toks_l, evs_l, cells_l = [], [], []
                ent_tok0 = f * TOKF + int(HBOFF[k0]) * 128
                rel0 = 0
                for hb in range(nb):
                    selh = hb_of == k0 + hb
                    cnt = int(selh.sum())
                    if cnt > TCAP[k0 + hb] * 128:
                        ok = False
                        break
                    toks_l.append(ent_tok0 + rel0 + np.arange(cnt))
                    evs_l.append(eidx[selh])
                    cells_l.append(ce[selh])
                    rel0 += TCAP[k0 + hb] * 128
                if not ok:
                    break
                toks = np.concatenate(toks_l)
                evs = np.concatenate(evs_l)
                cells_s = np.concatenate(cells_l)
                tp, tc = toks % 128, toks // 128
                coords[tp, tc * 4 + 0] = cb[evs, 0]
                coords[tp, tc * 4 + 1] = cb[evs, 1]
                coords[tp, tc * 4 + 2] = cb[evs, 2]
                coords[tp, tc * 4 + 3] = (cells_s - k0 * HB).astype(np.float32)
                # gather indices per call
                trel_all = toks - ent_tok0
                ent_c0 = k0 * HB
                ent_c1 = (k0 + nb) * HB - 1
                for (l, h) in _entry_calls(k0, nb):
                    ny, nt = NY[l], NT[l]
                    if h == -1:
                        tlo = 0
                        thi = sum(TCAP[k0:k0 + nb]) * 128
                        band = _cells_band(l, ent_c0, ent_c1)
                        ncol = (thi - tlo) // 16
                    else:
                        tlo = sum(TCAP[k0:k0 + h]) * 128
                        thi = tlo + TCAP[k0 + h] * 128
                        band = _cells_band(l, (k0 + h) * HB, (k0 + h + 1) * HB - 1)
                        ncol = (thi - tlo) // 16
                    inr = (trel_all >= tlo) & (trel_all < thi)
                    if inr.any():
                        evr = evs[inr]
                        bx = base[evr, l, 0] - _ixbase(core, l)
                        by = base[evr, l, 1]
                        bt = base[evr, l, 2]
                        loc = (bx * ny + by) * nt + bt - band
                        if loc.min() < 0 or loc.max() >= BANDR:
                            ok = False
                        trel = trel_all[inr] - tlo
                        idx16[trel % 16, cc + trel // 16] = loc.astype(np.int16)
                    cc += ncol
                if not ok:
                    break
            if not ok:
                break
        if not ok:
            return None
        cores.append({"coords": coords,
                      "idx16": np.tile(idx16, (8, 1))})
    return cores


def _repack_weights(inp):
    """conv/MLP weights into lhsT layouts."""
    w1 = np.asarray(inp["conv1_w"], np.float32)   # (256, 32, 3, 3)
    w2 = np.asarray(inp["conv2_w"], np.float32)   # (256, 256, 3, 3)
    w3 = np.asarray(inp["conv3_w"], np.float32)
    w4 = np.asarray(inp["conv4_w"], np.float32)   # (128, 256, 3, 3)

    km1 = np.zeros((32, 18 * 128), np.float32)
    for a in range(3):
        for b_ in range(3):
            ab = a * 3 + b_
            for oc in range(2):
                km1[:, (ab * 2 + oc) * 128:(ab * 2 + oc + 1) * 128] = \
                    w1[oc * 128:(oc + 1) * 128, :, a, b_].T
    import ml_dtypes
    km1 = km1.astype(ml_dtypes.bfloat16)
    def pack_big(w, nco):
        # -> [nco, 128, 18*128]: [oc][i][(ab*2+kc)*128+o]
        out = np.zeros((nco, 128, 18 * 128), np.float32)
        for a in range(3):
            for b_ in range(3):
                ab = a * 3 + b_
                for kc in range(2):
                    for oc in range(nco):
                        out[oc, :, (ab * 2 + kc) * 128:(ab * 2 + kc + 1) * 128] = \
                            w[oc * 128:(oc + 1) * 128, kc * 128:(kc + 1) * 128, a, b_].T
        return out
    km2 = pack_big(w2, 2)
    km3 = pack_big(w3, 2)
    km4 = pack_big(w4, 1)[0]      # [128, 2304]

    return {
        "km1": km1, "km2": km2, "km3": km3, "km4": km4,
        "cb1": np.asarray(inp["conv1_b"], np.float32),
        "cb2": np.asarray(inp["conv2_b"], np.float32),
        "cb3": np.asarray(inp["conv3_b"], np.float32),
        "cb4": np.asarray(inp["conv4_b"], np.float32),
        "mw0": np.asarray(inp["mlp0_w"], np.float32),
        "mw1": np.asarray(inp["mlp1_w"], np.float32),
        "mw2": np.asarray(inp["mlp2_w"], np.float32),
        "mw3": np.asarray(inp["mlp3_w"], np.float32),
        "mb0": np.asarray(inp["mlp0_b"], np.float32),
        "mb1": np.asarray(inp["mlp1_b"], np.float32),
        "mb2": np.asarray(inp["mlp2_b"], np.float32),
        "mb3": np.asarray(inp["mlp3_b"], np.float32),
    }


# ---------------------------------------------------------------- walrus fix

def _fix_walrus_wait_limit(nc):
    """This walrus build rejects >1 sem wait on most instructions. Hoist
    extra waits onto same-engine NoOp carriers (EventSemaphore excluded:
    its waits are event monitors, not engine-blocking)."""
    import concourse.mybir as mybir
    n_fixed = 0
    for fn in nc.m.functions:
        for bb in fn.blocks:
            changed = False
            new_insts = []
            for inst in bb.instructions:
                si = inst.sync_info
                if type(inst).__name__ == "InstEventSemaphore":
                    new_insts.append(inst)
                    continue
                if si is not None and len(si.on_wait) > 1:
                    waits = list(si.on_wait)
                    for w in waits[:-1]:
                        eng = nc.engines[inst.engine]
                        bi = eng.nop()
                        carrier = bi.ins
                        cur = nc.cur_bb.bb
                        lst = cur.instructions
                        assert lst and lst[-1].name == carrier.name
                        cur.instructions = lst[:-1]
                        carrier.sync_info = mybir.SyncInfo(on_wait=[w], on_update=[])
                        new_insts.append(carrier)
                    si.on_wait = waits[-1:]
                    changed = True
                    n_fixed += 1
                new_insts.append(inst)
            if changed:
                bb.instructions = new_insts
    return n_fixed


# ---------------------------------------------------------------- device IR

_PROG_CACHE = {}


def _build_program():
    key = tuple(TCAP)
    if key in _PROG_CACHE:
        return _PROG_CACHE[key]
    import concourse.bass as bass
    import concourse.bacc as bacc
    import concourse.mybir as mybir
    import concourse.tile as tile

    f32, i16, i32 = mybir.dt.float32, mybir.dt.int16, mybir.dt.int32
    bf16 = mybir.dt.bfloat16
    f32r = mybir.dt.float32r
    AF = mybir.ActivationFunctionType
    OP = mybir.AluOpType

    nc = bacc.Bacc("TRN2", target_bir_lowering=False, debug=False)

    coords = nc.declare_dram_parameter("coords", [128, (NTOK // 128) * 4], f32, isOutput=False)
    idx16 = nc.declare_dram_parameter("idx16", [128, IDXCOLS_ALL], i16, isOutput=False)
    expt = nc.declare_dram_parameter("expt", [TOTROW, EST], f32, isOutput=False)
    resc = nc.declare_dram_parameter("resc", [128, 24], f32, isOutput=False)
    emaskd = nc.declare_dram_parameter("emask", [128, 8], f32, isOutput=False)
    iotad = nc.declare_dram_parameter("iotad", [128, SLABMAX], f32, isOutput=False)
    km1d = nc.declare_dram_parameter("km1", [32, 2304], bf16, isOutput=False)
    km2d = nc.declare_dram_parameter("km2", [2, 128, 2304], f32r, isOutput=False)
    km3d = nc.declare_dram_parameter("km3", [2, 128, 2304], f32r, isOutput=False)
    km4d = nc.declare_dram_parameter("km4", [128, 2304], f32r, isOutput=False)
    cb1d = nc.declare_dram_parameter("cb1", [256], f32, isOutput=False)
    cb2d = nc.declare_dram_parameter("cb2", [256], f32, isOutput=False)
    cb3d = nc.declare_dram_parameter("cb3", [256], f32, isOutput=False)
    cb4d = nc.declare_dram_parameter("cb4", [128], f32, isOutput=False)
    mw0d = nc.declare_dram_parameter("mw0", [128, 128], f32r, isOutput=False)
    mw1d = nc.declare_dram_parameter("mw1", [128, 64], f32r, isOutput=False)
    mw2d = nc.declare_dram_parameter("mw2", [64, 32], f32r, isOutput=False)
    mw3d = nc.declare_dram_parameter("mw3", [32, 20], f32r, isOutput=False)
    mb0d = nc.declare_dram_parameter("mb0", [128], f32, isOutput=False)
    mb1d = nc.declare_dram_parameter("mb1", [64], f32, isOutput=False)
    mb2d = nc.declare_dram_parameter("mb2", [32], f32, isOutput=False)
    mb3d = nc.declare_dram_parameter("mb3", [20], f32, isOutput=False)
    outd = nc.declare_dram_parameter("out", [20, OUTPX], f32, isOutput=True)
    DBG = bool(int(__import__("os").environ.get("KDBG", "0")))
    if DBG:
        dbgd = nc.declare_dram_parameter("dbg_x0", [32, 3 * X0C], f32, isOutput=True)
        dbgy = nc.declare_dram_parameter("dbg_y", [128, 2 * NLW * TT], f32, isOutput=True)
        dbgy4 = nc.declare_dram_parameter("dbg_y4", [128, 2 * NLW * TT], f32, isOutput=True)

    def fap(tap, off, dims):
        return bass.AP(tap.tensor, tap.offset + off, [list(tap.ap[0])] + [list(d) for d in dims])

    def pslice(tap, p0, pn, off, dims):
        p = list(tap.ap[0])
        newp = [p[0], pn]
        return bass.AP(tap.tensor, tap.offset + p0 * p[0] + off, [newp] + [list(d) for d in dims])

    with tile.TileContext(nc) as tc:
        with nc.allow_low_precision(reason="fp32r matmul operands (same fp32 bits)"), \
             tc.tile_pool(name="const", bufs=1) as cp, \
             tc.tile_pool(name="wstream", bufs=2) as wsp, \
             tc.tile_pool(name="idxp", bufs=3) as ixp, \
             tc.tile_pool(name="vt", bufs=3) as vtp, \
             tc.tile_pool(name="enc", bufs=1) as ep, \
             tc.tile_pool(name="sp", bufs=3) as sp_, \
             tc.tile_pool(name="x0p", bufs=2) as xp, \
             tc.tile_pool(name="yp", bufs=2) as yp, \
             tc.tile_pool(name="ost", bufs=2) as osp, \
             tc.tile_pool(name="cpsum", bufs=3, space="PSUM") as cps, \
             tc.tile_pool(name="spsum", bufs=2, space="PSUM") as sps, \
             tc.tile_pool(name="mpsum", bufs=2, space="PSUM") as mps:

            # ---------------- constants
            coords_t = cp.tile([128, (NTOK // 128) * 4], f32)
            nc.sync.dma_start(out=coords_t[:], in_=coords[:])
            resc_t = cp.tile([128, 24], f32)
            nc.sync.dma_start(out=resc_t[:], in_=resc[:])
            emask_t = cp.tile([128, 8], f32)
            nc.sync.dma_start(out=emask_t[:], in_=emaskd[:])
            iota_t = cp.tile([128, SLABMAX], f32)
            nc.sync.dma_start(out=iota_t[:], in_=iotad[:])
            zero_t = cp.tile([128, 1], f32)
            nc.vector.memset(zero_t[:], 0.0)
            one_t = cp.tile([128, 1], f32)
            nc.vector.memset(one_t[:], 1.0)
            km1_t = cp.tile([32, 2304], bf16)
            nc.sync.dma_start(out=km1_t[:], in_=km1d[:])
            km4_t = cp.tile([128, 2304], f32r)
            nc.sync.dma_start(out=km4_t[:], in_=km4d[:])
            mw0_t = cp.tile([128, 128], f32r)
            nc.sync.dma_start(out=mw0_t[:], in_=mw0d[:])
            mw1_t = cp.tile([128, 64], f32r)
            nc.sync.dma_start(out=mw1_t[:], in_=mw1d[:])
            mw2_t = cp.tile([64, 32], f32r)
            nc.sync.dma_start(out=mw2_t[:], in_=mw2d[:])
            mw3_t = cp.tile([32, 20], f32r)
            nc.sync.dma_start(out=mw3_t[:], in_=mw3d[:])

            def bias_tile(dram, o0, n):
                t = cp.tile([n, 1], f32, tag=f"bias{dram.name}{o0}")
                nc.sync.dma_start(out=t[:], in_=dram[o0:o0 + n, None])
                return t
            cb1_t = [bias_tile(cb1d, o * 128, 128) for o in range(2)]
            cb2_t = [bias_tile(cb2d, o * 128, 128) for o in range(2)]
            cb3_t = [bias_tile(cb3d, o * 128, 128) for o in range(2)]
            cb4_t = [bias_tile(cb4d, 0, 128)]
            mb0_t = bias_tile(mb0d, 0, 128)
            mb1_t = bias_tile(mb1d, 0, 64)
            mb2_t = bias_tile(mb2d, 0, 32)
            mb3_t = bias_tile(mb3d, 0, 20)

            x0_tiles = {}

            # ---------------- encode one frame
            def encode_frame(f):
                x0_t = xp.tile([32, 3 * X0C], bf16, tag="x0")
                nc.vector.memset(x0_t[:], 0.0)
                x0_tiles[f] = x0_t
                cc_base = f * (IDXCOLS_ALL // B)
                for (k0, nb) in SLABS:
                    tps = sum(TCAP[k0:k0 + nb])       # tiles in this entry (<=12)
                    ncells = nb * HB
                    tcol0 = f * TILESF + int(HBOFF[k0])
                    ncols_entry = tps * 8 * 7 + tps * 8
                    ix_t = ixp.tile([128, TPSMAX * 8 * 8], i16, tag="ix")
                    nc.sync.dma_start(
                        out=ix_t[:, :ncols_entry],
                        in_=idx16[:, cc_base:cc_base + ncols_entry])
                    cc_base += ncols_entry
                    vt = vtp.tile([128, L * TPSMAX * EST], f32, tag="vt")
                    cc = 0
                    for (l, h) in _entry_calls(k0, nb):
                        if h == -1:
                            band = _cells_band(l, k0 * HB, (k0 + nb) * HB - 1)
                            ntok_c = tps * 128
                            vt_off = l * tps * EST
                        else:
                            band = _cells_band(l, (k0 + h) * HB, (k0 + h + 1) * HB - 1)
                            ntok_c = TCAP[k0 + h] * 128
                            vt_off = 7 * tps * EST + sum(TCAP[k0:k0 + h]) * EST
                        rstart = int(LBASE[l]) + band
                        nrow = min(BANDR, int(LBASE[l + 1]) - rstart)
                        nc.gpsimd.dma_gather(
                            out_ap=fap(vt[:], vt_off, [[EST, ntok_c // 128], [1, EST]]),
                            in_ap=expt[rstart:rstart + nrow, :],
                            idxs_ap=ix_t[:, cc:cc + ntok_c // 16],
                            num_idxs=ntok_c, num_idxs_reg=ntok_c,
                            elem_size=EST, single_packet=False)
                        cc += ntok_c // 16

                    n3 = L * tps * 3
                    scaled = ep.tile([128, L * TPSMAX * 3], f32, tag="scaled")
                    nc.vector.tensor_tensor(
                        out=scaled[:, :n3],
                        in0=fap(coords_t[:], tcol0 * 4, [[0, L], [4, tps], [1, 3]]),
                        in1=fap(resc_t[:], 0, [[3, L], [0, tps], [1, 3]]),
                        op=OP.mult)
                    ci = ep.tile([128, L * TPSMAX * 3], i32, tag="ci")
                    nc.vector.tensor_copy(out=ci[:, :n3], in_=scaled[:, :n3])
                    cf = ep.tile([128, L * TPSMAX * 3], f32, tag="cf")
                    nc.vector.tensor_copy(out=cf[:, :n3], in_=ci[:, :n3])
                    f0 = ep.tile([128, L * TPSMAX * 3], f32, tag="f0")
                    nc.vector.tensor_tensor(out=f0[:, :n3], in0=scaled[:, :n3], in1=cf[:, :n3], op=OP.subtract)
                    neg = ep.tile([128, L * TPSMAX * 3], f32, tag="neg")
                    nc.vector.tensor_tensor(out=neg[:, :n3], in0=f0[:, :n3], in1=zero_t[:].to_broadcast([128, n3]), op=OP.is_lt)
                    frac = ep.tile([128, L * TPSMAX * 3], f32, tag="frac")
                    nc.vector.tensor_tensor(out=frac[:, :n3], in0=f0[:, :n3], in1=neg[:, :n3], op=OP.add)
                    # F2 = (1-frac, frac) interleaved  (l, g, d, 2)
                    F2 = ep.tile([128, L * TPSMAX * 6], f32, tag="F2")
                    F2S = [[6 * tps, L], [6, tps], [2, 3]]
                    SH = [[3 * tps, L], [3, tps], [1, 3]]
                    nc.vector.tensor_tensor(
                        out=fap(F2[:], 0, F2S),
                        in0=one_t[:].to_broadcast([128, L, tps, 3]),
                        in1=fap(frac[:], 0, SH), op=OP.subtract)
                    nc.vector.tensor_copy(out=fap(F2[:], 1, F2S), in_=fap(frac[:], 0, SH))
                    # wxy (l, g, cy, cx)
                    wxy = ep.tile([128, L * TPSMAX * 4], f32, tag="wxy")
                    nc.vector.tensor_tensor(
                        out=wxy[:, :L * tps * 4],
                        in0=fap(F2[:], 0, [[6 * tps, L], [6, tps], [0, 2], [1, 2]]),
                        in1=fap(F2[:], 2, [[6 * tps, L], [6, tps], [1, 2], [0, 2]]),
                        op=OP.mult)
                    # w8 (l, g, ct, cy, cx)
                    w8 = ep.tile([128, L * TPSMAX * 8], f32, tag="w8")
                    nc.vector.tensor_tensor(
                        out=w8[:, :L * tps * 8],
                        in0=fap(wxy[:], 0, [[4 * tps, L], [4, tps], [0, 2], [1, 4]]),
                        in1=fap(F2[:], 4, [[6 * tps, L], [6, tps], [1, 2], [0, 4]]),
                        op=OP.mult)
                    # wv = vt * w8   (l, g, f, c)
                    wv = ep.tile([128, L * TPSMAX * 32], f32, tag="wv")
                    nc.vector.tensor_tensor(
                        out=wv[:, :L * tps * 32],
                        in0=fap(vt[:], 0, [[tps * EST, L], [EST, tps], [8, F], [1, 8]]),
                        in1=fap(w8[:], 0, [[8 * tps, L], [8, tps], [0, F], [1, 8]]),
                        op=OP.mult)
                    # encp in tile-major (g, l, f) order so the per-tile
                    # matmul lhsT slice is one contiguous 32-wide free dim
                    encp = ep.tile([128, L * TPSMAX * 4], f32, tag="encp")
                    with nc.allow_low_precision(reason="f32r matmul operand (fp32 bits)"):
                        nc.vector.tensor_reduce(
                            out=fap(encp[:], 0, [[4, L], [32, tps], [1, 4]]).bitcast(f32r),
                            in_=wv[:, :L * tps * 32].rearrange("p (a c) -> p a c", c=8),
                            op=OP.add, axis=mybir.AxisListType.X)
                    # one-hot scatter matmuls into the slab psum
                    ps = sps.tile([32, SLABMAX], f32, tag="sps")
                    for tt in range(tps):
                        S_t = sp_.tile([128, SLABMAX], f32r, tag="S")
                        nc.vector.tensor_tensor(
                            out=S_t[:, :ncells].bitcast(f32r),
                            in0=fap(coords_t[:], (tcol0 + tt) * 4 + 3, [[0, ncells]]),
                            in1=iota_t[:, :ncells],
                            op=OP.is_equal)
                        nc.tensor.matmul(
                            ps[:, :ncells],
                            lhsT=fap(encp[:], tt * 32, [[1, 32]]).bitcast(f32r),
                            rhs=S_t[:, :ncells].bitcast(f32r),
                            start=(tt == 0), stop=(tt == tps - 1))
                    # evict slab psum -> x0 (lw-run segments)
                    c0s = k0 * HB
                    c1s = c0s + ncells
                    cpos = c0s
                    while cpos < c1s:
                        lw = cpos // H
                        ce = min(c1s, (lw + 1) * H)
                        lwb = lw + 1
                        q, lwq = lwb % 3, lwb // 3
                        u0 = (cpos - lw * H) + 1
                        nc.scalar.activation(
                            out=fap(x0_t[:], q * X0C + lwq * (H + 2) + u0, [[1, ce - cpos]]),
                            in_=ps[:, cpos - c0s:ce - c0s],
                            func=AF.Copy)
                        cpos = ce
                    yield

            # ---------------- conv + mlp one frame
            def mask_edges(yt):
                # zero the out-of-frame lwb columns (core-dependent via the
                # emask input): lwb 1..4 <- emask[0:4], lwb 45..48 <- [4:8]
                for oc in range(2):
                    for (lwb0, m0) in ((1, 0), (45, 4)):
                        ap = fap(yt[:], oc * NLW * TT + lwb0 * TT, [[TT, 4], [1, TT]])
                        nc.vector.tensor_tensor(
                            out=ap, in0=ap,
                            in1=fap(emask_t[:], m0, [[1, 4], [0, TT]]),
                            op=OP.mult)

            KCONV = __import__("os").environ.get("KCONV", "mlp")

            def _dump_dbg(yt):
                # route a slice of the tile to outd so the phase has output
                nc.sync.dma_start(out=outd[:, :2650],
                                  in_=pslice(yt[:], 0, 20, 0, [[1, 2650]]).bitcast(f32))

            def conv_frame(f):
                x0_t = x0_tiles[f]
                for s in range(NS):
                    cy0 = CY0[s]
                    ws = _strip_windows(cy0)
                    t1lo, t1hi = ws[1]
                    y1 = yp.tile([128, 2 * NLW * TT], f32r, tag="y")
                    nc.vector.memset(y1[:].bitcast(f32), 0.0)
                    # conv1: K=32, x0 packed by q
                    for oc in range(2):
                        for q in range(3):
                            for half in range(3):
                                lws = [lwb for lwb in range(1, 49) if lwb % 3 == q][half * 6:(half + 1) * 6]
                                if not lws:
                                    continue
                                n_lw = len(lws)
                                ps = cps.tile([128, CPSW], f32, tag="cps")
                                nlen = t1hi - t1lo
                                first = True
                                for ab in range(9):
                                    dw, dh = ab // 3, ab % 3
                                    qin = (lws[0] + dw - 1) % 3
                                    lwqin = (lws[0] + dw - 1) // 3
                                    rhs = fap(x0_t[:],
                                              qin * X0C + lwqin * (H + 2) + cy0 + t1lo + dh,
                                              [[H + 2, n_lw], [1, nlen]])
                                    nc.tensor.matmul(
                                        ps[:, :n_lw * nlen],
                                        lhsT=km1_t[:, (ab * 2 + oc) * 128:(ab * 2 + oc + 1) * 128],
                                        rhs=rhs,
                                        start=first, stop=(ab == 8))
                                    first = False
                                # evict with relu+bias into y1 (strided by 4 lw)
                                nc.scalar.activation(
                                    out=fap(y1[:], oc * NLW * TT + lws[0] * TT + t1lo,
                                            [[3 * TT, n_lw], [1, nlen]]),
                                    in_=fap(ps[:], 0, [[nlen, n_lw], [1, nlen]]),
                                    func=AF.Relu, bias=cb1_t[oc][:])
                    mask_edges(y1)
                    yield
                    if KCONV == "c1":
                        if f == 0 and s == 0:
                            _dump_dbg(y1)
                        continue
                    # conv2 / conv3
                    prev = y1
                    for layer, (kmd, cbt) in enumerate([(km2d, cb2_t), (km3d, cb3_t)]):
                        lo, hi = ws[2 + layer]
                        nlen = hi - lo
                        ynext = yp.tile([128, 2 * NLW * TT], f32r, tag="y")
                        nc.vector.memset(ynext[:].bitcast(f32), 0.0)
                        for oc in range(2):
                            wt = wsp.tile([128, 2304], f32r, tag="w")
                            nc.scalar.dma_start(out=wt[:], in_=kmd[oc])
                            for g in range(8):
                                lw0 = 1 + 6 * g
                                n_lw = 6
                                ps = cps.tile([128, CPSW], f32, tag="cps")
                                idx = 0
                                for ab in range(9):
                                    dw, dh = ab // 3, ab % 3
                                    for kc in range(2):
                                        rhs = fap(prev[:], kc * NLW * TT + (lw0 + dw - 1) * TT + lo + dh - 1,
                                                  [[TT, n_lw], [1, nlen]])
                                        nc.tensor.matmul(
                                            ps[:, :n_lw * nlen],
                                            lhsT=wt[:, (ab * 2 + kc) * 128:(ab * 2 + kc + 1) * 128].bitcast(f32r),
                                            rhs=rhs.bitcast(f32r),
                                            start=(idx == 0), stop=(idx == 17))
                                        idx += 1
                                nc.scalar.activation(
                                    out=fap(ynext[:], oc * NLW * TT + lw0 * TT + lo,
                                            [[TT, n_lw], [1, nlen]]),
                                    in_=fap(ps[:], 0, [[nlen, n_lw], [1, nlen]]),
                                    func=AF.Relu, bias=cbt[oc][:])
                        mask_edges(ynext)
                        prev = ynext
                        yield
                        if KCONV == "c2" and layer == 0:
                            break
                        if DBG and f == 0 and s == 0 and layer == 0:
                            nc.sync.dma_start(out=dbgy[:], in_=ynext[:].bitcast(f32))
                    if KCONV in ("c2", "c3"):
                        if f == 0 and s == 0:
                            _dump_dbg(prev)
                        continue
                    # conv4 (oc=1)
                    lo4, hi4 = ws[4]
                    nlen = hi4 - lo4
                    y4 = yp.tile([128, 2 * NLW * TT], f32r, tag="y")
                    if DBG:
                        nc.vector.memset(y4[:].bitcast(f32), 0.0)
                    for g in range(8):
                        lw0 = 1 + 6 * g
                        n_lw = 6
                        ps = cps.tile([128, CPSW], f32, tag="cps")
                        idx = 0
                        for ab in range(9):
                            dw, dh = ab // 3, ab % 3
                            for kc in range(2):
                                rhs = fap(prev[:], kc * NLW * TT + (lw0 + dw - 1) * TT + lo4 + dh - 1,
                                          [[TT, n_lw], [1, nlen]])
                                nc.tensor.matmul(
                                    ps[:, :n_lw * nlen],
                                    lhsT=km4_t[:, (ab * 2 + kc) * 128:(ab * 2 + kc + 1) * 128].bitcast(f32r),
                                    rhs=rhs.bitcast(f32r),
                                    start=(idx == 0), stop=(idx == 17))
                                idx += 1
                        nc.scalar.activation(
                            out=fap(y4[:], lw0 * TT + lo4, [[TT, n_lw], [1, nlen]]),
                            in_=fap(ps[:], 0, [[nlen, n_lw], [1, nlen]]),
                            func=AF.Relu, bias=cb4_t[0][:])
                    yield
                    if DBG and f == 0 and s == 0:
                        nc.sync.dma_start(out=dbgy4[:], in_=y4[:].bitcast(f32))
                    if KCONV == "c4":
                        if f == 0 and s == 0:
                            _dump_dbg(y4)
                        continue
                    # mlp over y4's 46 computed rows; output rows t in
                    # [MLO, MLO+45) picked out at the DMA (skip = MLO-lo4)
                    MR = SROWS + 1                     # 46, even, 8*46=368>=256
                    skip = MLO - lo4
                    for j in range(5):
                        lw0 = 5 + 8 * j
                        rhs0 = fap(y4[:], lw0 * TT + lo4, [[TT, 8], [1, MR]])
                        p0 = mps.tile([128, 8 * MR], f32, tag="mp")
                        nc.tensor.matmul(p0[:], lhsT=mw0_t[:].bitcast(f32r), rhs=rhs0.bitcast(f32r), start=True, stop=True)
                        h1 = osp.tile([128, 8 * MR], f32r, tag="h1")
                        nc.scalar.activation(out=h1[:], in_=p0[:], func=AF.Relu, bias=mb0_t[:])
                        p1 = mps.tile([64, 8 * MR], f32, tag="mp")
                        nc.tensor.matmul(p1[:], lhsT=mw1_t[:].bitcast(f32r), rhs=h1[:].bitcast(f32r), start=True, stop=True)
                        h2 = osp.tile([64, 8 * MR], f32r, tag="h2")
                        nc.scalar.activation(out=h2[:], in_=p1[:], func=AF.Relu, bias=mb1_t[:])
                        p2 = mps.tile([32, 8 * MR], f32, tag="mp")
                        nc.tensor.matmul(p2[:], lhsT=mw2_t[:].bitcast(f32r), rhs=h2[:].bitcast(f32r), start=True, stop=True)
                        h3 = osp.tile([32, 8 * MR], f32r, tag="h3")
                        nc.scalar.activation(out=h3[:], in_=p2[:], func=AF.Relu, bias=mb2_t[:])
                        p3 = mps.tile([20, 8 * MR], f32, tag="mp")
                        nc.tensor.matmul(p3[:], lhsT=mw3_t[:].bitcast(f32r), rhs=h3[:].bitcast(f32r), start=True, stop=True)
                        ho = osp.tile([20, 8 * MR], f32, tag="ho")
                        nc.vector.tensor_tensor(out=ho[:], in0=p3[:], in1=mb3_t[:].to_broadcast([20, 8 * MR]), op=OP.add)
                        nc.scalar.dma_start(
                            out=fap(outd[:], (f * WS + (lw0 - 5)) * H + SROWS * s, [[H, 8], [1, SROWS]]),
                            in_=fap(ho[:], skip, [[MR, 8], [1, SROWS]]))
                    yield

            KPHASE = __import__("os").environ.get("KPHASE", "all")
            def _zero_x0(f):
                t = xp.tile([32, 3 * X0C], bf16, tag="x0")
                nc.vector.memset(t[:], 0.0)
                x0_tiles[f] = t

            if KPHASE == "all":
                for _ in encode_frame(0):
                    pass
                # interleave conv(0) strips with encode(1) entries so the
                # frame-1 gathers run while the PE does the frame-0 convs
                ge = encode_frame(1)
                gc = conv_frame(0)
                nsteps = NS * 5          # conv yield points
                per = max(1, len(SLABS) // nsteps)
                while next(gc, "done") != "done":
                    for _ in range(per):
                        if next(ge, "done") == "done":
                            break
                for _ in ge:
                    pass
                for _ in conv_frame(1):
                    pass
            elif KPHASE == "enc":
                for f in range(B):
                    for _ in encode_frame(f):
                        pass
            else:
                for f in range(B):
                    _zero_x0(f)
                    for _ in conv_frame(f):
                        pass
            if DBG:
                dx = cp.tile([32, 3 * X0C], f32, tag="dbgx")
                nc.vector.tensor_copy(out=dx[:], in_=x0_tiles[0][:])
                nc.sync.dma_start(out=dbgd[:], in_=dx[:])
            if KPHASE == "enc":
                zo = cp.tile([20, OUTPX], f32)
                nc.vector.memset(zo[:], 0.0)
                nc.vector.tensor_tensor(
                    out=zo[:20, :96], in0=zo[:20, :96],
                    in1=pslice(x0_tiles[0][:], 0, 20, 0, [[1, 96]]), op=OP.add)
                nc.sync.dma_start(out=outd[:], in_=zo[:])

    nc.compile()
    import os as _os
    if _os.environ.get("NO_WAITFIX", "0") != "1":
        _fix_walrus_wait_limit(nc)
    _PROG_CACHE[key] = nc
    return nc


EDGE_LWB = [1, 2, 3, 4, 45, 46, 47, 48]


def _edge_mask(core):
    """[128, 8] column mask: 0.0 where the edge lwb is out of the frame."""
    cx0 = WS * core - HALO
    m = np.array([1.0 if 0 <= cx0 + (lwb - 1) < W else 0.0 for lwb in EDGE_LWB],
                 dtype=np.float32)
    return np.tile(m[None, :], (128, 1))


# ---------------------------------------------------------------- numpy ref

def _numpy_forward(inp):
    """Fallback faithful forward in numpy (slow)."""
    cb = np.asarray(inp["currentBlock"], np.float32)
    ec = np.asarray(inp["eventCounts"]).astype(np.int64)
    ht = np.asarray(inp["hash_tables"], np.float32)
    n = cb.shape[0]
    scaled = cb[:, None, :] * RES[None].astype(np.float32)
    basef = np.floor(scaled)
    frac = scaled - basef
    base = basef.astype(np.uint32)
    enc = np.zeros((n, L, F), np.float32)
    for c in range(8):
        off = np.array([(c >> d) & 1 for d in range(3)], np.uint32)
        idx = base + off[None, None, :]
        hsh = ((idx[..., 0] * PRIMES[0]) ^ (idx[..., 1] * PRIMES[1]) ^ (idx[..., 2] * PRIMES[2])) % TBL
        w = np.where(off.astype(bool)[None, None, :], frac, 1.0 - frac).prod(-1)
        for l in range(L):
            enc[:, l, :] += w[:, l, None] * ht[l][hsh[:, l].astype(np.int64)]
    enc = enc.reshape(n, L * F)
    bidx = np.repeat(np.arange(B), ec)
    cx = np.clip(np.round(cb[:, 0] * np.float32(W)), 0, W - 1).astype(np.int64)
    cy = np.clip(np.round(cb[:, 1] * np.float32(H)), 0, H - 1).astype(np.int64)
    ff = np.zeros((B, W, H, L * F), np.float32)
    np.add.at(ff, (bidx, cx, cy), enc)
    x = ff.transpose(0, 3, 1, 2)
    def conv(x, w, b):
        Bn, Ci, Wn, Hn = x.shape
        Co = w.shape[0]
        y = np.zeros((Bn, Co, Wn, Hn), np.float32)
        xp = np.pad(x, ((0, 0), (0, 0), (1, 1), (1, 1)))
        for a in range(3):
            for bb in range(3):
                y += np.einsum("oi,biwh->bowh", w[:, :, a, bb],
                               xp[:, :, a:a + Wn, bb:bb + Hn], optimize=True)
        return y + b[None, :, None, None]
    x = np.maximum(conv(x, np.asarray(inp["conv1_w"], np.float32), np.asarray(inp["conv1_b"], np.float32)), 0)
    x = np.maximum(conv(x, np.asarray(inp["conv2_w"], np.float32), np.asarray(inp["conv2_b"], np.float32)), 0)
    x = np.maximum(conv(x, np.asarray(inp["conv3_w"], np.float32), np.asarray(inp["conv3_b"], np.float32)), 0)
    x = np.maximum(conv(x, np.asarray(inp["conv4_w"], np.float32), np.asarray(inp["conv4_b"], np.float32)), 0)
    x = x.transpose(0, 2, 3, 1)
    h1 = np.maximum(x @ np.asarray(inp["mlp0_w"], np.float32) + np.asarray(inp["mlp0_b"], np.float32), 0)
    h2 = np.maximum(h1 @ np.asarray(inp["mlp1_w"], np.float32) + np.asarray(inp["mlp1_b"], np.float32), 0)
    h3 = np.maximum(h2 @ np.asarray(inp["mlp2_w"], np.float32) + np.asarray(inp["mlp2_b"], np.float32), 0)
    return (h3 @ np.asarray(inp["mlp3_w"], np.float32) + np.asarray(inp["mlp3_b"], np.float32)).astype(np.float32)


# ---------------------------------------------------------------- entry

def kernel(**inputs):
    try:
        return _device_kernel(**inputs)
    except Exception as e:  # device/compile failure: stay correct
        print(f"kernel: device path failed ({type(e).__name__}: {e}); numpy fallback")
        return _numpy_forward(inputs)


def _ensure_axon():
    """The caller may have initialized jax on cpu (e.g. to run the jax
    reference). The device run needs the axon backend: reset if needed."""
    import jax
    try:
        devs = jax.devices()
        if len(devs) >= NCORES and "cpu" not in str(devs[0]).lower():
            return
    except Exception:
        pass
    try:
        jax.config.update("jax_platforms", "axon")
    except Exception:
        pass
    try:
        jax.clear_backends()
    except Exception:
        pass
    try:
        from jax._src import xla_bridge as _xb
        _xb._clear_backends()
    except Exception:
        pass
    devs = jax.devices()
    assert len(devs) >= NCORES, f"need {NCORES} devices, got {devs}"


def _fit_caps(currentBlock, eventCounts):
    """Per-bucket tile capacities sized to the actual event distribution
    (max over cores/frames, +1 tile headroom cap at tile granularity)."""
    cb = np.asarray(currentBlock, dtype=np.float32)
    ec = np.asarray(eventCounts).astype(np.int64)
    n = cb.shape[0]
    bidx = np.repeat(np.arange(B, dtype=np.int64), ec)
    if bidx.shape[0] != n:
        bidx = np.resize(bidx, n)
    cx = np.clip(np.round(cb[:, 0] * np.float32(W)), 0, W - 1).astype(np.int64)
    cy = np.clip(np.round(cb[:, 1] * np.float32(H)), 0, H - 1).astype(np.int64)
    caps = np.zeros(NHB, dtype=np.int64)
    for core in range(NCORES):
        cx0 = WS * core - HALO
        sel = (cx >= cx0) & (cx < cx0 + WL)
        cell = (cx[sel] - cx0) * H + cy[sel]
        hb = cell // HB + bidx[sel] * NHB
        cnt = np.bincount(hb, minlength=2 * NHB).reshape(2, NHB).max(axis=0)
        caps = np.maximum(caps, cnt)
    return [int(-(-c // 128)) for c in caps]


def _device_kernel(**inputs):
    _ensure_axon()
    _set_caps(_fit_caps(inputs["currentBlock"], inputs["eventCounts"]))
    ev = _prep_events(inputs["currentBlock"], inputs["eventCounts"])
    if ev is None:
        print("kernel: slab capacity exceeded; numpy fallback")
        return _numpy_forward(inputs)

    from concourse.bass_utils import run_bass_kernel_spmd

    nc = _build_program()
    expts = _expand_tables(inputs["hash_tables"])
    wts = _repack_weights(inputs)

    resc = np.zeros((128, 24), np.float32)
    resc[:, :] = RES.astype(np.float32).reshape(-1)[None, :]
    iota = np.tile(np.arange(SLABMAX, dtype=np.float32)[None, :], (128, 1))

    shared = {"resc": resc, "iotad": iota,
              "km1": wts["km1"], "km2": wts["km2"], "km3": wts["km3"], "km4": wts["km4"],
              "cb1": wts["cb1"], "cb2": wts["cb2"], "cb3": wts["cb3"], "cb4": wts["cb4"],
              "mw0": wts["mw0"], "mw1": wts["mw1"], "mw2": wts["mw2"], "mw3": wts["mw3"],
              "mb0": wts["mb0"], "mb1": wts["mb1"], "mb2": wts["mb2"], "mb3": wts["mb3"]}
    in_maps = []
    for core in range(NCORES):
        m = dict(shared)
        m["coords"] = ev[core]["coords"]
        m["idx16"] = ev[core]["idx16"]
        m["expt"] = expts[core]
        m["emask"] = _edge_mask(core)
        in_maps.append(m)

    res = run_bass_kernel_spmd(nc, in_maps, list(range(NCORES)))

    out = np.zeros((B, W, H, T), np.float32)
    for core in range(NCORES):
        o = res.results[core]["out"]           # [20, OUTPX]
        o = o.reshape(T, B, WS, H)
        out[:, WS * core:WS * (core + 1), :, :] = o.transpose(1, 2, 3, 0)
    return out



# revision 13
# speedup vs baseline: 1.9451x; 1.9451x over previous
"""Trainium2 Bass kernel for nn_EventPixelFF.

Pipeline (per NeuronCore, 8-way sharded over W columns):
  - host: shard events by output column (with conv halo), sort by output
    cell, bucket into 480-cell slabs; compute gather indices into a
    per-core pre-expanded hash table (8 corners baked per cell, feature-
    major); repack conv/MLP weights into lhsT layout.
  - device: trilinear weights on DVE, table rows via gpsimd dma_gather,
    weighted corner-reduce on DVE, scatter into the feature grid via
    one-hot matmuls on PE (PSUM-accumulated per slab), then the conv
    stack + per-pixel MLP as fp32r matmuls with shifted-view rhs.
"""

import sys
import numpy as np

for _p in ("/opt/trn_rl_repo", "/root/.axon_site/_ro/trn_rl_repo"):
    if _p not in sys.path:
        sys.path.insert(0, _p)

# ---------------------------------------------------------------- constants
L = 8
F = 4
TBL = 1 << 19
W, H = 320, 180
T = 20
B = 2
C0 = 128
NCORES = 8
WS = 40          # output W columns per core
HALO = 4
WL = WS + 2 * HALO   # 48 local columns with halo

# resolutions exactly as reference._resolutions() computes them (float64
# log/exp path, margins ~1e-6 -- safe to hardcode)
RES = np.array([[16, 16, 4], [24, 22, 5], [37, 31, 8], [57, 45, 11],
                [88, 63, 16], [135, 90, 24], [208, 127, 34], [320, 179, 50]],
               dtype=np.int32)
PRIMES = np.array([1, 2654435761, 805459861], dtype=np.uint32)

NY = [int(RES[l][1]) + 1 for l in range(L)]
NT = [int(RES[l][2]) + 1 for l in range(L)]

# per-core local table x-extent. For core with first local column at global
# cx0: events at local column lw have global ix = floor(x*rx) with
# x*320 in [cx0+lw-0.5, cx0+lw+0.5] =>
#   ix in [floor((2*(cx0+lw)-1)*rx/640), floor((2*(cx0+lw)+1)*rx/640)]
# (exact integer arithmetic; +-1 f32 safety added except rx=320 where the
# relation is exact because the same f32 product feeds round and floor).

def _ix_min(cxg, l):
    rx = int(RES[l][0])
    v = ((2 * cxg - 1) * rx) // 640
    return v if rx == 320 else v - 1


def _ix_max(cxg, l):
    rx = int(RES[l][0])
    v = ((2 * cxg + 1) * rx) // 640
    return v if rx == 320 else v + 1


def _ixbase(core, l):
    cx0 = WS * core - HALO
    return _ix_min(cx0, l)


IXW = []
for l in range(L):
    w = 0
    for core in range(NCORES):
        cx0 = WS * core - HALO
        w = max(w, _ix_max(cx0 + WL - 1, l) + 1 - _ixbase(core, l) + 1)
    IXW.append(w)
LBASE = np.concatenate([[0], np.cumsum([IXW[l] * NY[l] * NT[l] for l in range(L)])]).astype(np.int64)
TOTROW = int(LBASE[-1])
EST = 64  # elements per expanded-table row (32 used + pad to 256B)

# token geometry: 240-cell half-buckets; image-edge columns (cx=0/319, i.e.
# local lw=4 on core 0 and lw=47 on core 7) catch 1.5x density from clipping,
# so buckets 3 and 35 get extra capacity.
HB = 240
NHB = (WL * H) // HB            # 36 half-buckets per frame
PAD_CELLREL = 10000.0
BANDR = 32768  # int16 index reach
SLABMAX = HB      # one-hot width (240)

# slab entries: one 240-cell bucket each (nb=2 slabs fail HW execution
# on this runtime -- redacted INTERNAL error; keep nb=1)
SLABS = [(k, 1) for k in range(NHB)]

# gather calls per slab entry: levels 0..6 whole entry; level 7 per bucket
def _entry_calls(k0, nb):
    calls = [(l, -1) for l in range(7)]
    for hb in range(nb):
        calls.append((7, hb) if nb > 1 else (7, -1))
    return calls


NQ = 4  # SWDGE queues (ucode max); gathers round-robin across them


def _ncalls():
    return B * sum(len(_entry_calls(k0, nb)) for (k0, nb) in SLABS)


def _set_caps(tcap):
    """(Re)derive the token geometry from per-bucket tile capacities."""
    global TCAP, HBOFF, TILESF, TOKF, NTOK, TPSMAX, IDXCOLS_ALL
    TCAP = list(tcap)
    HBOFF = np.concatenate([[0], np.cumsum(TCAP)]).astype(np.int64)
    TILESF = int(HBOFF[-1])
    TOKF = TILESF * 128
    NTOK = B * TOKF
    TPSMAX = max(sum(TCAP[k0:k0 + nb]) for (k0, nb) in SLABS)
    IDXCOLS_ALL = B * sum(
        (sum(TCAP[k0:k0 + nb]) * 8) * 7 + sum(TCAP[k0 + hb] * 8 for hb in range(nb))
        for (k0, nb) in SLABS)


_set_caps([8 if k in (3, 35) else 6 for k in range(NHB)])

# conv strip geometry. Strip s computes output rows cy in [45s, 45s+45)
# at window t = cy - cy0 with cy0 = 45s - 4. Each conv layer shrinks the
# computed window by one row per side (validity cascade); windows are
# clamped to the image so out-of-image rows stay memset-zero (= conv
# 'SAME' padding), and lengths forced even (fp32r matmul ISA rule). When
# parity needs fixing: if the bottom is image-clamped, shrink the top
# (the lost top row only feeds outputs that are discarded); else extend
# down one junk row that no later layer reads.
NS = 4
SROWS = H // NS                  # 45 output rows per strip
TT = SROWS + 8                   # 53 rows per strip window
CY0 = [SROWS * s - 4 for s in range(NS)]
MLO = 4                          # output rows are t in [4, 49)


def _strip_windows(cy0):
    lo1 = max(1, -cy0)
    hi1 = min(TT, H - cy0)
    if (hi1 - lo1) % 2:
        if -cy0 >= 1:
            hi1 -= 1
        else:
            lo1 -= 1
    ws = {1: (lo1, hi1)}
    for k, (bl, bh) in ((2, (2, 52)), (3, (3, 51)), (4, (4, 50))):
        lo = max(bl, -cy0)
        hi = min(bh, H - cy0)
        if (hi - lo) % 2:
            if -cy0 >= bl:
                hi -= 1
            else:
                lo -= 1
        ws[k] = (lo, hi)
    assert ws[4][1] - ws[4][0] == SROWS + 1, (cy0, ws)
    return ws


NLW = 50                         # lw blocks incl +-1 pads (real lw at block lw+1)
LWQ = 17                         # ceil(50/3) : x0 packed [f+32*(lwb%3), (lwb//3)*182+u]
X0C = LWQ * (H + 2) + 2          # 3094 cols + 2 pad
CPSW = 6 * 52                    # conv psum tile width (max 6 lws x 52 rows)

OUTPX = B * WS * H               # 14400


def _band_from_lws(l, lwlo, lwhi):
    """core-independent band start: min over cores of the local ix lower
    bound for columns [lwlo, lwhi]; asserts the worst-case width fits."""
    lo = min(_ix_min(WS * core - HALO + lwlo, l) - _ixbase(core, l)
             for core in range(NCORES))
    hi = max(_ix_max(WS * core - HALO + lwhi, l) - _ixbase(core, l)
             for core in range(NCORES))
    lo = max(0, lo)
    assert (hi - lo + 1) * NY[l] * NT[l] <= BANDR, (l, lwlo, lwhi, lo, hi)
    start = lo * NY[l] * NT[l]
    maxstart = max(0, IXW[l] * NY[l] * NT[l] - BANDR)
    return min(start, maxstart)


def _cells_band(l, c0, c1):
    return _band_from_lws(l, c0 // H, c1 // H)


# ---------------------------------------------------------------- host prep

def _expand_tables(hash_tables):
    """Build the per-core-family expanded tables. Returns list of 8 arrays
    [TOTROW, EST] float32 (one per core; they differ by x-slice)."""
    ht = np.asarray(hash_tables, dtype=np.float32)  # (L, TBL, F)
    out = []
    for core in range(NCORES):
        exp = np.zeros((TOTROW, EST), dtype=np.float32)
        cx0 = WS * core - HALO  # global cx of local lw=0
        for l in range(L):
            rx = int(RES[l][0])
            ny, nt = NY[l], NT[l]
            ixbase = _ixbase(core, l)
            ixs = ixbase + np.arange(IXW[l])              # global ix values
            iys = np.arange(ny)
            its = np.arange(nt)
            IX, IY, IT = np.meshgrid(ixs, iys, its, indexing="ij")
            rows = np.zeros((IXW[l], ny, nt, F, 8), dtype=np.float32)
            for c in range(8):
                ox, oy, ot = c & 1, (c >> 1) & 1, (c >> 2) & 1
                a = (IX + ox).astype(np.int64)
                # clip negatives/overflow: events never index there, value irrelevant
                a = np.clip(a, 0, None).astype(np.uint32)
                b = (IY + oy).astype(np.uint32)
                d = (IT + ot).astype(np.uint32)
                hsh = ((a * PRIMES[0]) ^ (b * PRIMES[1]) ^ (d * PRIMES[2])) % TBL
                rows[:, :, :, :, c] = ht[l][hsh.astype(np.int64)]
            exp[LBASE[l]:LBASE[l + 1], :32] = rows.reshape(-1, 32)
        out.append(exp)
    return out


def _prep_events(currentBlock, eventCounts):
    """Shard + sort + slab-bucket events. Returns per-core dicts with
    coords [128, 4*NTOK/128] f32 and idx16 [128, IDXCOLS_ALL] i16,
    or None if any slab overflows its fixed capacity."""
    cb = np.asarray(currentBlock, dtype=np.float32)
    ec = np.asarray(eventCounts).astype(np.int64)
    n = cb.shape[0]
    bidx = np.repeat(np.arange(B, dtype=np.int64), ec, )
    if bidx.shape[0] != n:
        # pad/truncate defensively (counts should sum to N)
        bidx = np.resize(bidx, n)
    x, y = cb[:, 0], cb[:, 1]
    cx = np.clip(np.round(x * np.float32(W)), 0, W - 1).astype(np.int64)
    cy = np.clip(np.round(y * np.float32(H)), 0, H - 1).astype(np.int64)

    # per (event, level) local cell ids (host-side integer planning)
    scaled = cb[:, None, :] * RES[None, :, :].astype(np.float32)   # (n, L, 3)
    base = np.floor(scaled).astype(np.int64)                        # (n, L, 3)

    cores = []
    for core in range(NCORES):
        cx0 = WS * core - HALO
        sel = (cx >= cx0) & (cx < cx0 + WL)
        ei = np.nonzero(sel)[0]
        lw = cx[ei] - cx0
        cell = lw * H + cy[ei]
        fr = bidx[ei]
        order = np.lexsort((cell, fr))
        ei, lw, cell, fr = ei[order], lw[order], cell[order], fr[order]

        coords = np.full((128, (NTOK // 128) * 4), 0.5, dtype=np.float32)
        coords[:, 3::4] = PAD_CELLREL
        # -1 = "skip" (trailing negative idxs generate no DMA descriptors);
        # each call's valid prefix is padded to a 16-multiple with idx 0.
        idx16 = np.full((16, IDXCOLS_ALL), -1, dtype=np.int16)
        gcnt = np.zeros(_ncalls(), dtype=np.int32)
        gidx = 0

        ok = True
        for f in range(B):
            m = fr == f
            ce = cell[m]
            eidx = ei[m]
            hb_of = ce // HB
            cc = f * (IDXCOLS_ALL // B)
            for (k0, nb) in SLABS:
                # place events of each bucket into its fixed tile range
                toks_l, evs_l, cells_l = [], [], []
                ent_tok0 = f * TOKF + int(HBOFF[k0]) * 128
                rel0 = 0
                for hb in range(nb):
                    selh = hb_of == k0 + hb
                    cnt = int(selh.sum())
                    if cnt > TCAP[k0 + hb] * 128:
                        ok = False
                        break
                    toks_l.append(ent_tok0 + rel0 + np.arange(cnt))
                    evs_l.append(eidx[selh])
                    cells_l.append(ce[selh])
                    rel0 += TCAP[k0 + hb] * 128
                if not ok:
                    break
                toks = np.concatenate(toks_l)
                evs = np.concatenate(evs_l)
                cells_s = np.concatenate(cells_l)
                tp, tc = toks % 128, toks // 128
                coords[tp, tc * 4 + 0] = cb[evs, 0]
                coords[tp, tc * 4 + 1] = cb[evs, 1]
                coords[tp, tc * 4 + 2] = cb[evs, 2]
                coords[tp, tc * 4 + 3] = (cells_s - k0 * HB).astype(np.float32)
                # gather indices per call
                trel_all = toks - ent_tok0
                ent_c0 = k0 * HB
                ent_c1 = (k0 + nb) * HB - 1
                for (l, h) in _entry_calls(k0, nb):
                    ny, nt = NY[l], NT[l]
                    if h == -1:
                        tlo = 0
                        thi = sum(TCAP[k0:k0 + nb]) * 128
                        band = _cells_band(l, ent_c0, ent_c1)
                        ncol = (thi - tlo) // 16
                    else:
                        tlo = sum(TCAP[k0:k0 + h]) * 128
                        thi = tlo + TCAP[k0 + h] * 128
                        band = _cells_band(l, (k0 + h) * HB, (k0 + h + 1) * HB - 1)
                        ncol = (thi - tlo) // 16
                    inr = (trel_all >= tlo) & (trel_all < thi)
                    cntc = int(inr.sum())
                    if cntc:
                        evr = evs[inr]
                        bx = base[evr, l, 0] - _ixbase(core, l)
                        by = base[evr, l, 1]
                        bt = base[evr, l, 2]
                        loc = (bx * ny + by) * nt + bt - band
                        if loc.min() < 0 or loc.max() >= BANDR:
                            ok = False
                        trel = trel_all[inr] - tlo
                        idx16[trel % 16, cc + trel // 16] = loc.astype(np.int16)
                    nvalid = max(16, -(-cntc // 16) * 16)
                    if nvalid > cntc:
                        tp_ = np.arange(cntc, nvalid)
                        idx16[tp_ % 16, cc + tp_ // 16] = 0
                    gcnt[gidx] = nvalid
                    gidx += 1
                    cc += ncol
                if not ok:
                    break
            if not ok:
                break
        if not ok:
            return None
        cores.append({"coords": coords,
                      "idx16": np.tile(idx16, (8, 1)),
                      "gcnt": np.tile(gcnt[None, :], (16, 1))})
    return cores


def _repack_weights(inp):
    """conv/MLP weights into lhsT layouts."""
    w1 = np.asarray(inp["conv1_w"], np.float32)   # (256, 32, 3, 3)
    w2 = np.asarray(inp["conv2_w"], np.float32)   # (256, 256, 3, 3)
    w3 = np.asarray(inp["conv3_w"], np.float32)
    w4 = np.asarray(inp["conv4_w"], np.float32)   # (128, 256, 3, 3)

    km1 = np.zeros((32, 18 * 128), np.float32)
    for a in range(3):
        for b_ in range(3):
            ab = a * 3 + b_
            for oc in range(2):
                km1[:, (ab * 2 + oc) * 128:(ab * 2 + oc + 1) * 128] = \
                    w1[oc * 128:(oc + 1) * 128, :, a, b_].T
    import ml_dtypes
    km1 = km1.astype(ml_dtypes.bfloat16)
    def pack_big(w, nco):
        # -> [nco, 128, 18*128]: [oc][i][(ab*2+kc)*128+o]
        out = np.zeros((nco, 128, 18 * 128), np.float32)
        for a in range(3):
            for b_ in range(3):
                ab = a * 3 + b_
                for kc in range(2):
                    for oc in range(nco):
                        out[oc, :, (ab * 2 + kc) * 128:(ab * 2 + kc + 1) * 128] = \
                            w[oc * 128:(oc + 1) * 128, kc * 128:(kc + 1) * 128, a, b_].T
        return out
    km2 = pack_big(w2, 2)
    km3 = pack_big(w3, 2)
    km4 = pack_big(w4, 1)[0]      # [128, 2304]

    return {
        "km1": km1, "km2": km2, "km3": km3, "km4": km4,
        "cb1": np.asarray(inp["conv1_b"], np.float32),
        "cb2": np.asarray(inp["conv2_b"], np.float32),
        "cb3": np.asarray(inp["conv3_b"], np.float32),
        "cb4": np.asarray(inp["conv4_b"], np.float32),
        "mw0": np.asarray(inp["mlp0_w"], np.float32),
        "mw1": np.asarray(inp["mlp1_w"], np.float32),
        "mw2": np.asarray(inp["mlp2_w"], np.float32),
        "mw3": np.asarray(inp["mlp3_w"], np.float32),
        "mb0": np.asarray(inp["mlp0_b"], np.float32),
        "mb1": np.asarray(inp["mlp1_b"], np.float32),
        "mb2": np.asarray(inp["mlp2_b"], np.float32),
        "mb3": np.asarray(inp["mlp3_b"], np.float32),
    }


# ---------------------------------------------------------------- walrus fix

def _fix_walrus_wait_limit(nc):
    """This walrus build rejects >1 sem wait on most instructions. Hoist
    extra waits onto same-engine NoOp carriers (EventSemaphore excluded:
    its waits are event monitors, not engine-blocking)."""
    import concourse.mybir as mybir
    n_fixed = 0
    for fn in nc.m.functions:
        for bb in fn.blocks:
            changed = False
            new_insts = []
            for inst in bb.instructions:
                si = inst.sync_info
                if type(inst).__name__ == "InstEventSemaphore":
                    new_insts.append(inst)
                    continue
                if si is not None and len(si.on_wait) > 1:
                    waits = list(si.on_wait)
                    for w in waits[:-1]:
                        eng = nc.engines[inst.engine]
                        bi = eng.nop()
                        carrier = bi.ins
                        cur = nc.cur_bb.bb
                        lst = cur.instructions
                        assert lst and lst[-1].name == carrier.name
                        cur.instructions = lst[:-1]
                        carrier.sync_info = mybir.SyncInfo(on_wait=[w], on_update=[])
                        new_insts.append(carrier)
                    si.on_wait = waits[-1:]
                    changed = True
                    n_fixed += 1
                new_insts.append(inst)
            if changed:
                bb.instructions = new_insts
    return n_fixed


# ---------------------------------------------------------------- device IR

_PROG_CACHE = {}


def _build_program():
    key = tuple(TCAP)
    if key in _PROG_CACHE:
        return _PROG_CACHE[key]
    import concourse.bass as bass
    import concourse.bacc as bacc
    import concourse.mybir as mybir
    import concourse.tile as tile

    f32, i16, i32 = mybir.dt.float32, mybir.dt.int16, mybir.dt.int32
    bf16 = mybir.dt.bfloat16
    f32r = mybir.dt.float32r
    AF = mybir.ActivationFunctionType
    OP = mybir.AluOpType

    nc = bacc.Bacc("TRN2", target_bir_lowering=False, debug=False,
                   num_swdge_queues=NQ)

    coords = nc.declare_dram_parameter("coords", [128, (NTOK // 128) * 4], f32, isOutput=False)
    idx16 = nc.declare_dram_parameter("idx16", [128, IDXCOLS_ALL], i16, isOutput=False)
    gcntd = nc.declare_dram_parameter("gcnt", [16, _ncalls()], i32, isOutput=False)
    expt = nc.declare_dram_parameter("expt", [TOTROW, EST], f32, isOutput=False)
    resc = nc.declare_dram_parameter("resc", [128, 24], f32, isOutput=False)
    emaskd = nc.declare_dram_parameter("emask", [128, 8], f32, isOutput=False)
    iotad = nc.declare_dram_parameter("iotad", [128, SLABMAX], f32, isOutput=False)
    km1d = nc.declare_dram_parameter("km1", [32, 2304], bf16, isOutput=False)
    km2d = nc.declare_dram_parameter("km2", [2, 128, 2304], f32r, isOutput=False)
    km3d = nc.declare_dram_parameter("km3", [2, 128, 2304], f32r, isOutput=False)
    km4d = nc.declare_dram_parameter("km4", [128, 2304], f32r, isOutput=False)
    cb1d = nc.declare_dram_parameter("cb1", [256], f32, isOutput=False)
    cb2d = nc.declare_dram_parameter("cb2", [256], f32, isOutput=False)
    cb3d = nc.declare_dram_parameter("cb3", [256], f32, isOutput=False)
    cb4d = nc.declare_dram_parameter("cb4", [128], f32, isOutput=False)
    mw0d = nc.declare_dram_parameter("mw0", [128, 128], f32r, isOutput=False)
    mw1d = nc.declare_dram_parameter("mw1", [128, 64], f32r, isOutput=False)
    mw2d = nc.declare_dram_parameter("mw2", [64, 32], f32r, isOutput=False)
    mw3d = nc.declare_dram_parameter("mw3", [32, 20], f32r, isOutput=False)
    mb0d = nc.declare_dram_parameter("mb0", [128], f32, isOutput=False)
    mb1d = nc.declare_dram_parameter("mb1", [64], f32, isOutput=False)
    mb2d = nc.declare_dram_parameter("mb2", [32], f32, isOutput=False)
    mb3d = nc.declare_dram_parameter("mb3", [20], f32, isOutput=False)
    outd = nc.declare_dram_parameter("out", [20, OUTPX], f32, isOutput=True)
    DBG = bool(int(__import__("os").environ.get("KDBG", "0")))
    if DBG:
        dbgd = nc.declare_dram_parameter("dbg_x0", [32, 3 * X0C], f32, isOutput=True)
        dbgy = nc.declare_dram_parameter("dbg_y", [128, 2 * NLW * TT], f32, isOutput=True)
        dbgy4 = nc.declare_dram_parameter("dbg_y4", [128, 2 * NLW * TT], f32, isOutput=True)

    def fap(tap, off, dims):
        return bass.AP(tap.tensor, tap.offset + off, [list(tap.ap[0])] + [list(d) for d in dims])

    def pslice(tap, p0, pn, off, dims):
        p = list(tap.ap[0])
        newp = [p[0], pn]
        return bass.AP(tap.tensor, tap.offset + p0 * p[0] + off, [newp] + [list(d) for d in dims])

    with tile.TileContext(nc) as tc:
        with nc.allow_low_precision(reason="fp32r matmul operands (same fp32 bits)"), \
             tc.tile_pool(name="const", bufs=1) as cp, \
             tc.tile_pool(name="wstream", bufs=2) as wsp, \
             tc.tile_pool(name="idxp", bufs=3) as ixp, \
             tc.tile_pool(name="vt", bufs=3) as vtp, \
             tc.tile_pool(name="enc", bufs=1) as ep, \
             tc.tile_pool(name="sp", bufs=3) as sp_, \
             tc.tile_pool(name="x0p", bufs=2) as xp, \
             tc.tile_pool(name="yp", bufs=2) as yp, \
             tc.tile_pool(name="ost", bufs=2) as osp, \
             tc.tile_pool(name="cpsum", bufs=3, space="PSUM") as cps, \
             tc.tile_pool(name="spsum", bufs=2, space="PSUM") as sps, \
             tc.tile_pool(name="mpsum", bufs=2, space="PSUM") as mps:

            # ---------------- constants
            coords_t = cp.tile([128, (NTOK // 128) * 4], f32)
            nc.sync.dma_start(out=coords_t[:], in_=coords[:])
            gcnt_t = cp.tile([16, _ncalls()], i32)
            nc.sync.dma_start(out=gcnt_t[:], in_=gcntd[:])
            resc_t = cp.tile([128, 24], f32)
            nc.sync.dma_start(out=resc_t[:], in_=resc[:])
            emask_t = cp.tile([128, 8], f32)
            nc.sync.dma_start(out=emask_t[:], in_=emaskd[:])
            iota_t = cp.tile([128, SLABMAX], f32)
            nc.sync.dma_start(out=iota_t[:], in_=iotad[:])
            zero_t = cp.tile([128, 1], f32)
            nc.vector.memset(zero_t[:], 0.0)
            one_t = cp.tile([128, 1], f32)
            nc.vector.memset(one_t[:], 1.0)
            km1_t = cp.tile([32, 2304], bf16)
            nc.sync.dma_start(out=km1_t[:], in_=km1d[:])
            km4_t = cp.tile([128, 2304], f32r)
            nc.sync.dma_start(out=km4_t[:], in_=km4d[:])
            mw0_t = cp.tile([128, 128], f32r)
            nc.sync.dma_start(out=mw0_t[:], in_=mw0d[:])
            mw1_t = cp.tile([128, 64], f32r)
            nc.sync.dma_start(out=mw1_t[:], in_=mw1d[:])
            mw2_t = cp.tile([64, 32], f32r)
            nc.sync.dma_start(out=mw2_t[:], in_=mw2d[:])
            mw3_t = cp.tile([32, 20], f32r)
            nc.sync.dma_start(out=mw3_t[:], in_=mw3d[:])

            def bias_tile(dram, o0, n):
                t = cp.tile([n, 1], f32, tag=f"bias{dram.name}{o0}")
                nc.sync.dma_start(out=t[:], in_=dram[o0:o0 + n, None])
                return t
            cb1_t = [bias_tile(cb1d, o * 128, 128) for o in range(2)]
            cb2_t = [bias_tile(cb2d, o * 128, 128) for o in range(2)]
            cb3_t = [bias_tile(cb3d, o * 128, 128) for o in range(2)]
            cb4_t = [bias_tile(cb4d, 0, 128)]
            mb0_t = bias_tile(mb0d, 0, 128)
            mb1_t = bias_tile(mb1d, 0, 64)
            mb2_t = bias_tile(mb2d, 0, 32)
            mb3_t = bias_tile(mb3d, 0, 20)

            x0_tiles = {}

            # prime the vt ring: skipped (-1) gather slots keep stale SBUF
            # data, which must be finite (pad tokens have nonzero trilinear
            # weights; their one-hot scatter row is all-zero, so any finite
            # value is harmless -- NaN is not).
            for _ in range(3):
                vt0 = vtp.tile([128, L * TPSMAX * EST], f32, tag="vt")
                nc.vector.memset(vt0[:], 0.0)

            gcall = [0]
            # one shared Pool register for the per-call gather counts (a
            # fresh value_load per call exhausts Pool's register file; Pool
            # executes in order so reuse is WAR-safe)
            gcnt_reg = nc.gpsimd.alloc_register("gcnt_reg")

            # ---------------- encode one frame
            def encode_frame(f):
                x0_t = xp.tile([32, 3 * X0C], bf16, tag="x0")
                nc.vector.memset(x0_t[:], 0.0)
                x0_tiles[f] = x0_t
                cc_base = f * (IDXCOLS_ALL // B)
                for (k0, nb) in SLABS:
                    tps = sum(TCAP[k0:k0 + nb])       # tiles in this entry (<=12)
                    ncells = nb * HB
                    tcol0 = f * TILESF + int(HBOFF[k0])
                    ncols_entry = tps * 8 * 7 + tps * 8
                    ix_t = ixp.tile([128, TPSMAX * 8 * 8], i16, tag="ix")
                    nc.sync.dma_start(
                        out=ix_t[:, :ncols_entry],
                        in_=idx16[:, cc_base:cc_base + ncols_entry])
                    cc_base += ncols_entry
                    vt = vtp.tile([128, L * TPSMAX * EST], f32, tag="vt")
                    cc = 0
                    for (l, h) in _entry_calls(k0, nb):
                        if h == -1:
                            band = _cells_band(l, k0 * HB, (k0 + nb) * HB - 1)
                            ntok_c = tps * 128
                            vt_off = l * tps * EST
                        else:
                            band = _cells_band(l, (k0 + h) * HB, (k0 + h + 1) * HB - 1)
                            ntok_c = TCAP[k0 + h] * 128
                            vt_off = 7 * tps * EST + sum(TCAP[k0:k0 + h]) * EST
                        rstart = int(LBASE[l]) + band
                        nrow = min(BANDR, int(LBASE[l + 1]) - rstart)
                        gi = gcall[0]
                        gcall[0] += 1
                        nc.gpsimd.reg_load(gcnt_reg, gcnt_t[0:1, gi:gi + 1])
                        nc.gpsimd.dma_gather(
                            out_ap=fap(vt[:], vt_off, [[EST, ntok_c // 128], [1, EST]]),
                            in_ap=expt[rstart:rstart + nrow, :],
                            idxs_ap=ix_t[:, cc:cc + ntok_c // 16],
                            num_idxs=ntok_c, num_idxs_reg=gcnt_reg,
                            elem_size=EST, single_packet=False,
                            queue_num=gi % NQ)
                        cc += ntok_c // 16

                    n3 = L * tps * 3
                    scaled = ep.tile([128, L * TPSMAX * 3], f32, tag="scaled")
                    nc.vector.tensor_tensor(
                        out=scaled[:, :n3],
                        in0=fap(coords_t[:], tcol0 * 4, [[0, L], [4, tps], [1, 3]]),
                        in1=fap(resc_t[:], 0, [[3, L], [0, tps], [1, 3]]),
                        op=OP.mult)
                    ci = ep.tile([128, L * TPSMAX * 3], i32, tag="ci")
                    nc.vector.tensor_copy(out=ci[:, :n3], in_=scaled[:, :n3])
                    cf = ep.tile([128, L * TPSMAX * 3], f32, tag="cf")
                    nc.vector.tensor_copy(out=cf[:, :n3], in_=ci[:, :n3])
                    f0 = ep.tile([128, L * TPSMAX * 3], f32, tag="f0")
                    nc.vector.tensor_tensor(out=f0[:, :n3], in0=scaled[:, :n3], in1=cf[:, :n3], op=OP.subtract)
                    neg = ep.tile([128, L * TPSMAX * 3], f32, tag="neg")
                    nc.vector.tensor_tensor(out=neg[:, :n3], in0=f0[:, :n3], in1=zero_t[:].to_broadcast([128, n3]), op=OP.is_lt)
                    frac = ep.tile([128, L * TPSMAX * 3], f32, tag="frac")
                    nc.vector.tensor_tensor(out=frac[:, :n3], in0=f0[:, :n3], in1=neg[:, :n3], op=OP.add)
                    # F2 = (1-frac, frac) interleaved  (l, g, d, 2)
                    F2 = ep.tile([128, L * TPSMAX * 6], f32, tag="F2")
                    F2S = [[6 * tps, L], [6, tps], [2, 3]]
                    SH = [[3 * tps, L], [3, tps], [1, 3]]
                    nc.vector.tensor_tensor(
                        out=fap(F2[:], 0, F2S),
                        in0=one_t[:].to_broadcast([128, L, tps, 3]),
                        in1=fap(frac[:], 0, SH), op=OP.subtract)
                    nc.vector.tensor_copy(out=fap(F2[:], 1, F2S), in_=fap(frac[:], 0, SH))
                    # wxy (l, g, cy, cx)
                    wxy = ep.tile([128, L * TPSMAX * 4], f32, tag="wxy")
                    nc.vector.tensor_tensor(
                        out=wxy[:, :L * tps * 4],
                        in0=fap(F2[:], 0, [[6 * tps, L], [6, tps], [0, 2], [1, 2]]),
                        in1=fap(F2[:], 2, [[6 * tps, L], [6, tps], [1, 2], [0, 2]]),
                        op=OP.mult)
                    # w8 (l, g, ct, cy, cx)
                    w8 = ep.tile([128, L * TPSMAX * 8], f32, tag="w8")
                    nc.vector.tensor_tensor(
                        out=w8[:, :L * tps * 8],
                        in0=fap(wxy[:], 0, [[4 * tps, L], [4, tps], [0, 2], [1, 4]]),
                        in1=fap(F2[:], 4, [[6 * tps, L], [6, tps], [1, 2], [0, 4]]),
                        op=OP.mult)
                    # wv = vt * w8   (l, g, f, c)
                    wv = ep.tile([128, L * TPSMAX * 32], f32, tag="wv")
                    nc.vector.tensor_tensor(
                        out=wv[:, :L * tps * 32],
                        in0=fap(vt[:], 0, [[tps * EST, L], [EST, tps], [8, F], [1, 8]]),
                        in1=fap(w8[:], 0, [[8 * tps, L], [8, tps], [0, F], [1, 8]]),
                        op=OP.mult)
                    # encp in tile-major (g, l, f) order so the per-tile
                    # matmul lhsT slice is one contiguous 32-wide free dim
                    encp = ep.tile([128, L * TPSMAX * 4], f32, tag="encp")
                    with nc.allow_low_precision(reason="f32r matmul operand (fp32 bits)"):
                        nc.vector.tensor_reduce(
                            out=fap(encp[:], 0, [[4, L], [32, tps], [1, 4]]).bitcast(f32r),
                            in_=wv[:, :L * tps * 32].rearrange("p (a c) -> p a c", c=8),
                            op=OP.add, axis=mybir.AxisListType.X)
                    # one-hot scatter matmuls into the slab psum
                    ps = sps.tile([32, SLABMAX], f32, tag="sps")
                    for tt in range(tps):
                        S_t = sp_.tile([128, SLABMAX], f32r, tag="S")
                        nc.vector.tensor_tensor(
                            out=S_t[:, :ncells].bitcast(f32r),
                            in0=fap(coords_t[:], (tcol0 + tt) * 4 + 3, [[0, ncells]]),
                            in1=iota_t[:, :ncells],
                            op=OP.is_equal)
                        nc.tensor.matmul(
                            ps[:, :ncells],
                            lhsT=fap(encp[:], tt * 32, [[1, 32]]).bitcast(f32r),
                            rhs=S_t[:, :ncells].bitcast(f32r),
                            start=(tt == 0), stop=(tt == tps - 1))
                    # evict slab psum -> x0 (lw-run segments)
                    c0s = k0 * HB
                    c1s = c0s + ncells
                    cpos = c0s
                    while cpos < c1s:
                        lw = cpos // H
                        ce = min(c1s, (lw + 1) * H)
                        lwb = lw + 1
                        q, lwq = lwb % 3, lwb // 3
                        u0 = (cpos - lw * H) + 1
                        nc.scalar.activation(
                            out=fap(x0_t[:], q * X0C + lwq * (H + 2) + u0, [[1, ce - cpos]]),
                            in_=ps[:, cpos - c0s:ce - c0s],
                            func=AF.Copy)
                        cpos = ce
                    yield

            # ---------------- conv + mlp one frame
            def mask_edges(yt):
                # zero the out-of-frame lwb columns (core-dependent via the
                # emask input): lwb 1..4 <- emask[0:4], lwb 45..48 <- [4:8]
                for oc in range(2):
                    for (lwb0, m0) in ((1, 0), (45, 4)):
                        ap = fap(yt[:], oc * NLW * TT + lwb0 * TT, [[TT, 4], [1, TT]])
                        nc.vector.tensor_tensor(
                            out=ap, in0=ap,
                            in1=fap(emask_t[:], m0, [[1, 4], [0, TT]]),
                            op=OP.mult)

            KCONV = __import__("os").environ.get("KCONV", "mlp")

            def _dump_dbg(yt):
                # route a slice of the tile to outd so the phase has output
                nc.sync.dma_start(out=outd[:, :2650],
                                  in_=pslice(yt[:], 0, 20, 0, [[1, 2650]]).bitcast(f32))

            def conv_frame(f):
                x0_t = x0_tiles[f]
                for s in range(NS):
                    cy0 = CY0[s]
                    ws = _strip_windows(cy0)
                    t1lo, t1hi = ws[1]
                    y1 = yp.tile([128, 2 * NLW * TT], f32r, tag="y")
                    nc.vector.memset(y1[:].bitcast(f32), 0.0)
                    # conv1: K=32, x0 packed by q
                    for oc in range(2):
                        for q in range(3):
                            for half in range(3):
                                lws = [lwb for lwb in range(1, 49) if lwb % 3 == q][half * 6:(half + 1) * 6]
                                if not lws:
                                    continue
                                n_lw = len(lws)
                                ps = cps.tile([128, CPSW], f32, tag="cps")
                                nlen = t1hi - t1lo
                                first = True
                                for ab in range(9):
                                    dw, dh = ab // 3, ab % 3
                                    qin = (lws[0] + dw - 1) % 3
                                    lwqin = (lws[0] + dw - 1) // 3
                                    rhs = fap(x0_t[:],
                                              qin * X0C + lwqin * (H + 2) + cy0 + t1lo + dh,
                                              [[H + 2, n_lw], [1, nlen]])
                                    nc.tensor.matmul(
                                        ps[:, :n_lw * nlen],
                                        lhsT=km1_t[:, (ab * 2 + oc) * 128:(ab * 2 + oc + 1) * 128],
                                        rhs=rhs,
                                        start=first, stop=(ab == 8))
                                    first = False
                                # evict with relu+bias into y1 (strided by 4 lw)
                                nc.scalar.activation(
                                    out=fap(y1[:], oc * NLW * TT + lws[0] * TT + t1lo,
                                            [[3 * TT, n_lw], [1, nlen]]),
                                    in_=fap(ps[:], 0, [[nlen, n_lw], [1, nlen]]),
                                    func=AF.Relu, bias=cb1_t[oc][:])
                    mask_edges(y1)
                    yield
                    if KCONV == "c1":
                        if f == 0 and s == 0:
                            _dump_dbg(y1)
                        continue
                    # conv2 / conv3
                    prev = y1
                    for layer, (kmd, cbt) in enumerate([(km2d, cb2_t), (km3d, cb3_t)]):
                        lo, hi = ws[2 + layer]
                        nlen = hi - lo
                        ynext = yp.tile([128, 2 * NLW * TT], f32r, tag="y")
                        nc.vector.memset(ynext[:].bitcast(f32), 0.0)
                        for oc in range(2):
                            wt = wsp.tile([128, 2304], f32r, tag="w")
                            nc.scalar.dma_start(out=wt[:], in_=kmd[oc])
                            for g in range(8):
                                lw0 = 1 + 6 * g
                                n_lw = 6
                                ps = cps.tile([128, CPSW], f32, tag="cps")
                                idx = 0
                                for ab in range(9):
                                    dw, dh = ab // 3, ab % 3
                                    for kc in range(2):
                                        rhs = fap(prev[:], kc * NLW * TT + (lw0 + dw - 1) * TT + lo + dh - 1,
                                                  [[TT, n_lw], [1, nlen]])
                                        nc.tensor.matmul(
                                            ps[:, :n_lw * nlen],
                                            lhsT=wt[:, (ab * 2 + kc) * 128:(ab * 2 + kc + 1) * 128].bitcast(f32r),
                                            rhs=rhs.bitcast(f32r),
                                            start=(idx == 0), stop=(idx == 17))
                                        idx += 1
                                nc.scalar.activation(
                                    out=fap(ynext[:], oc * NLW * TT + lw0 * TT + lo,
                                            [[TT, n_lw], [1, nlen]]),
                                    in_=fap(ps[:], 0, [[nlen, n_lw], [1, nlen]]),
                                    func=AF.Relu, bias=cbt[oc][:])
                        mask_edges(ynext)
                        prev = ynext
                        yield
                        if KCONV == "c2" and layer == 0:
                            break
                        if DBG and f == 0 and s == 0 and layer == 0:
                            nc.sync.dma_start(out=dbgy[:], in_=ynext[:].bitcast(f32))
                    if KCONV in ("c2", "c3"):
                        if f == 0 and s == 0:
                            _dump_dbg(prev)
                        continue
                    # conv4 (oc=1)
                    lo4, hi4 = ws[4]
                    nlen = hi4 - lo4
                    y4 = yp.tile([128, 2 * NLW * TT], f32r, tag="y")
                    if DBG:
                        nc.vector.memset(y4[:].bitcast(f32), 0.0)
                    for g in range(8):
                        lw0 = 1 + 6 * g
                        n_lw = 6
                        ps = cps.tile([128, CPSW], f32, tag="cps")
                        idx = 0
                        for ab in range(9):
                            dw, dh = ab // 3, ab % 3
                            for kc in range(2):
                                rhs = fap(prev[:], kc * NLW * TT + (lw0 + dw - 1) * TT + lo4 + dh - 1,
                                          [[TT, n_lw], [1, nlen]])
                                nc.tensor.matmul(
                                    ps[:, :n_lw * nlen],
                                    lhsT=km4_t[:, (ab * 2 + kc) * 128:(ab * 2 + kc + 1) * 128].bitcast(f32r),
                                    rhs=rhs.bitcast(f32r),
                                    start=(idx == 0), stop=(idx == 17))
                                idx += 1
                        nc.scalar.activation(
                            out=fap(y4[:], lw0 * TT + lo4, [[TT, n_lw], [1, nlen]]),
                            in_=fap(ps[:], 0, [[nlen, n_lw], [1, nlen]]),
                            func=AF.Relu, bias=cb4_t[0][:])
                    yield
                    if DBG and f == 0 and s == 0:
                        nc.sync.dma_start(out=dbgy4[:], in_=y4[:].bitcast(f32))
                    if KCONV == "c4":
                        if f == 0 and s == 0:
                            _dump_dbg(y4)
                        continue
                    # mlp over y4's 46 computed rows; output rows t in
                    # [MLO, MLO+45) picked out at the DMA (skip = MLO-lo4)
                    MR = SROWS + 1                     # 46, even, 8*46=368>=256
                    skip = MLO - lo4
                    for j in range(5):
                        lw0 = 5 + 8 * j
                        rhs0 = fap(y4[:], lw0 * TT + lo4, [[TT, 8], [1, MR]])
                        p0 = mps.tile([128, 8 * MR], f32, tag="mp")
                        nc.tensor.matmul(p0[:], lhsT=mw0_t[:].bitcast(f32r), rhs=rhs0.bitcast(f32r), start=True, stop=True)
                        h1 = osp.tile([128, 8 * MR], f32r, tag="h1")
                        nc.scalar.activation(out=h1[:], in_=p0[:], func=AF.Relu, bias=mb0_t[:])
                        p1 = mps.tile([64, 8 * MR], f32, tag="mp")
                        nc.tensor.matmul(p1[:], lhsT=mw1_t[:].bitcast(f32r), rhs=h1[:].bitcast(f32r), start=True, stop=True)
                        h2 = osp.tile([64, 8 * MR], f32r, tag="h2")
                        nc.scalar.activation(out=h2[:], in_=p1[:], func=AF.Relu, bias=mb1_t[:])
                        p2 = mps.tile([32, 8 * MR], f32, tag="mp")
                        nc.tensor.matmul(p2[:], lhsT=mw2_t[:].bitcast(f32r), rhs=h2[:].bitcast(f32r), start=True, stop=True)
                        h3 = osp.tile([32, 8 * MR], f32r, tag="h3")
                        nc.scalar.activation(out=h3[:], in_=p2[:], func=AF.Relu, bias=mb2_t[:])
                        p3 = mps.tile([20, 8 * MR], f32, tag="mp")
                        nc.tensor.matmul(p3[:], lhsT=mw3_t[:].bitcast(f32r), rhs=h3[:].bitcast(f32r), start=True, stop=True)
                        ho = osp.tile([20, 8 * MR], f32, tag="ho")
                        nc.vector.tensor_tensor(out=ho[:], in0=p3[:], in1=mb3_t[:].to_broadcast([20, 8 * MR]), op=OP.add)
                        nc.scalar.dma_start(
                            out=fap(outd[:], (f * WS + (lw0 - 5)) * H + SROWS * s, [[H, 8], [1, SROWS]]),
                            in_=fap(ho[:], skip, [[MR, 8], [1, SROWS]]))
                    yield

            KPHASE = __import__("os").environ.get("KPHASE", "all")
            def _zero_x0(f):
                t = xp.tile([32, 3 * X0C], bf16, tag="x0")
                nc.vector.memset(t[:], 0.0)
                x0_tiles[f] = t

            if KPHASE == "all":
                for _ in encode_frame(0):
                    pass
                # interleave conv(0) strips with encode(1) entries so the
                # frame-1 gathers run while the PE does the frame-0 convs
                ge = encode_frame(1)
                gc = conv_frame(0)
                nsteps = NS * 5          # conv yield points
                per = max(1, len(SLABS) // nsteps)
                while next(gc, "done") != "done":
                    for _ in range(per):
                        if next(ge, "done") == "done":
                            break
                for _ in ge:
                    pass
                for _ in conv_frame(1):
                    pass
            elif KPHASE == "enc":
                for f in range(B):
                    for _ in encode_frame(f):
                        pass
            else:
                for f in range(B):
                    _zero_x0(f)
                    for _ in conv_frame(f):
                        pass
            if DBG:
                dx = cp.tile([32, 3 * X0C], f32, tag="dbgx")
                nc.vector.tensor_copy(out=dx[:], in_=x0_tiles[0][:])
                nc.sync.dma_start(out=dbgd[:], in_=dx[:])
            if KPHASE == "enc":
                zo = cp.tile([20, OUTPX], f32)
                nc.vector.memset(zo[:], 0.0)
                nc.vector.tensor_tensor(
                    out=zo[:20, :96], in0=zo[:20, :96],
                    in1=pslice(x0_tiles[0][:], 0, 20, 0, [[1, 96]]), op=OP.add)
                nc.sync.dma_start(out=outd[:], in_=zo[:])

    nc.compile()
    import os as _os
    if _os.environ.get("NO_WAITFIX", "0") != "1":
        _fix_walrus_wait_limit(nc)
    _PROG_CACHE[key] = nc
    return nc


EDGE_LWB = [1, 2, 3, 4, 45, 46, 47, 48]


def _edge_mask(core):
    """[128, 8] column mask: 0.0 where the edge lwb is out of the frame."""
    cx0 = WS * core - HALO
    m = np.array([1.0 if 0 <= cx0 + (lwb - 1) < W else 0.0 for lwb in EDGE_LWB],
                 dtype=np.float32)
    return np.tile(m[None, :], (128, 1))


# ---------------------------------------------------------------- numpy ref

def _numpy_forward(inp):
    """Fallback faithful forward in numpy (slow)."""
    cb = np.asarray(inp["currentBlock"], np.float32)
    ec = np.asarray(inp["eventCounts"]).astype(np.int64)
    ht = np.asarray(inp["hash_tables"], np.float32)
    n = cb.shape[0]
    scaled = cb[:, None, :] * RES[None].astype(np.float32)
    basef = np.floor(scaled)
    frac = scaled - basef
    base = basef.astype(np.uint32)
    enc = np.zeros((n, L, F), np.float32)
    for c in range(8):
        off = np.array([(c >> d) & 1 for d in range(3)], np.uint32)
        idx = base + off[None, None, :]
        hsh = ((idx[..., 0] * PRIMES[0]) ^ (idx[..., 1] * PRIMES[1]) ^ (idx[..., 2] * PRIMES[2])) % TBL
        w = np.where(off.astype(bool)[None, None, :], frac, 1.0 - frac).prod(-1)
        for l in range(L):
            enc[:, l, :] += w[:, l, None] * ht[l][hsh[:, l].astype(np.int64)]
    enc = enc.reshape(n, L * F)
    bidx = np.repeat(np.arange(B), ec)
    cx = np.clip(np.round(cb[:, 0] * np.float32(W)), 0, W - 1).astype(np.int64)
    cy = np.clip(np.round(cb[:, 1] * np.float32(H)), 0, H - 1).astype(np.int64)
    ff = np.zeros((B, W, H, L * F), np.float32)
    np.add.at(ff, (bidx, cx, cy), enc)
    x = ff.transpose(0, 3, 1, 2)
    def conv(x, w, b):
        Bn, Ci, Wn, Hn = x.shape
        Co = w.shape[0]
        y = np.zeros((Bn, Co, Wn, Hn), np.float32)
        xp = np.pad(x, ((0, 0), (0, 0), (1, 1), (1, 1)))
        for a in range(3):
            for bb in range(3):
                y += np.einsum("oi,biwh->bowh", w[:, :, a, bb],
                               xp[:, :, a:a + Wn, bb:bb + Hn], optimize=True)
        return y + b[None, :, None, None]
    x = np.maximum(conv(x, np.asarray(inp["conv1_w"], np.float32), np.asarray(inp["conv1_b"], np.float32)), 0)
    x = np.maximum(conv(x, np.asarray(inp["conv2_w"], np.float32), np.asarray(inp["conv2_b"], np.float32)), 0)
    x = np.maximum(conv(x, np.asarray(inp["conv3_w"], np.float32), np.asarray(inp["conv3_b"], np.float32)), 0)
    x = np.maximum(conv(x, np.asarray(inp["conv4_w"], np.float32), np.asarray(inp["conv4_b"], np.float32)), 0)
    x = x.transpose(0, 2, 3, 1)
    h1 = np.maximum(x @ np.asarray(inp["mlp0_w"], np.float32) + np.asarray(inp["mlp0_b"], np.float32), 0)
    h2 = np.maximum(h1 @ np.asarray(inp["mlp1_w"], np.float32) + np.asarray(inp["mlp1_b"], np.float32), 0)
    h3 = np.maximum(h2 @ np.asarray(inp["mlp2_w"], np.float32) + np.asarray(inp["mlp2_b"], np.float32), 0)
    return (h3 @ np.asarray(inp["mlp3_w"], np.float32) + np.asarray(inp["mlp3_b"], np.float32)).astype(np.float32)


# ---------------------------------------------------------------- entry

def kernel(**inputs):
    try:
        return _device_kernel(**inputs)
    except Exception as e:  # device/compile failure: stay correct
        print(f"kernel: device path failed ({type(e).__name__}: {e}); numpy fallback")
        return _numpy_forward(inputs)


def _ensure_axon():
    """The caller may have initialized jax on cpu (e.g. to run the jax
    reference). The device run needs the axon backend: reset if needed."""
    import jax
    try:
        devs = jax.devices()
        if len(devs) >= NCORES and "cpu" not in str(devs[0]).lower():
            return
    except Exception:
        pass
    try:
        jax.config.update("jax_platforms", "axon")
    except Exception:
        pass
    try:
        jax.clear_backends()
    except Exception:
        pass
    try:
        from jax._src import xla_bridge as _xb
        _xb._clear_backends()
    except Exception:
        pass
    devs = jax.devices()
    assert len(devs) >= NCORES, f"need {NCORES} devices, got {devs}"


def _fit_caps(currentBlock, eventCounts):
    """Per-bucket tile capacities sized to the actual event distribution
    (max over cores/frames, +1 tile headroom cap at tile granularity)."""
    cb = np.asarray(currentBlock, dtype=np.float32)
    ec = np.asarray(eventCounts).astype(np.int64)
    n = cb.shape[0]
    bidx = np.repeat(np.arange(B, dtype=np.int64), ec)
    if bidx.shape[0] != n:
        bidx = np.resize(bidx, n)
    cx = np.clip(np.round(cb[:, 0] * np.float32(W)), 0, W - 1).astype(np.int64)
    cy = np.clip(np.round(cb[:, 1] * np.float32(H)), 0, H - 1).astype(np.int64)
    caps = np.zeros(NHB, dtype=np.int64)
    for core in range(NCORES):
        cx0 = WS * core - HALO
        sel = (cx >= cx0) & (cx < cx0 + WL)
        cell = (cx[sel] - cx0) * H + cy[sel]
        hb = cell // HB + bidx[sel] * NHB
        cnt = np.bincount(hb, minlength=2 * NHB).reshape(2, NHB).max(axis=0)
        caps = np.maximum(caps, cnt)
    return [int(-(-c // 128)) for c in caps]


def _device_kernel(**inputs):
    _ensure_axon()
    _set_caps(_fit_caps(inputs["currentBlock"], inputs["eventCounts"]))
    ev = _prep_events(inputs["currentBlock"], inputs["eventCounts"])
    if ev is None:
        print("kernel: slab capacity exceeded; numpy fallback")
        return _numpy_forward(inputs)

    from concourse.bass_utils import run_bass_kernel_spmd

    nc = _build_program()
    expts = _expand_tables(inputs["hash_tables"])
    wts = _repack_weights(inputs)

    resc = np.zeros((128, 24), np.float32)
    resc[:, :] = RES.astype(np.float32).reshape(-1)[None, :]
    iota = np.tile(np.arange(SLABMAX, dtype=np.float32)[None, :], (128, 1))

    shared = {"resc": resc, "iotad": iota,
              "km1": wts["km1"], "km2": wts["km2"], "km3": wts["km3"], "km4": wts["km4"],
              "cb1": wts["cb1"], "cb2": wts["cb2"], "cb3": wts["cb3"], "cb4": wts["cb4"],
              "mw0": wts["mw0"], "mw1": wts["mw1"], "mw2": wts["mw2"], "mw3": wts["mw3"],
              "mb0": wts["mb0"], "mb1": wts["mb1"], "mb2": wts["mb2"], "mb3": wts["mb3"]}
    in_maps = []
    for core in range(NCORES):
        m = dict(shared)
        m["coords"] = ev[core]["coords"]
        m["idx16"] = ev[core]["idx16"]
        m["gcnt"] = ev[core]["gcnt"]
        m["expt"] = expts[core]
        m["emask"] = _edge_mask(core)
        in_maps.append(m)

    res = run_bass_kernel_spmd(nc, in_maps, list(range(NCORES)))

    out = np.zeros((B, W, H, T), np.float32)
    for core in range(NCORES):
        o = res.results[core]["out"]           # [20, OUTPX]
        o = o.reshape(T, B, WS, H)
        out[:, WS * core:WS * (core + 1), :, :] = o.transpose(1, 2, 3, 0)
    return out



# revision 24
# speedup vs baseline: 2.2738x; 1.1690x over previous
"""Trainium2 Bass kernel for nn_EventPixelFF.

Pipeline (per NeuronCore, 8-way sharded over W columns):
  - host: shard events by output column (with conv halo), sort by output
    cell, bucket into 480-cell slabs; compute gather indices into a
    per-core pre-expanded hash table (8 corners baked per cell, feature-
    major); repack conv/MLP weights into lhsT layout.
  - device: trilinear weights on DVE, table rows via gpsimd dma_gather,
    weighted corner-reduce on DVE, scatter into the feature grid via
    one-hot matmuls on PE (PSUM-accumulated per slab), then the conv
    stack + per-pixel MLP as fp32r matmuls with shifted-view rhs.
"""

import sys
import numpy as np

for _p in ("/opt/trn_rl_repo", "/root/.axon_site/_ro/trn_rl_repo"):
    if _p not in sys.path:
        sys.path.insert(0, _p)

# ---------------------------------------------------------------- constants
L = 8
F = 4
TBL = 1 << 19
W, H = 320, 180
T = 20
B = 2
C0 = 128
NCORES = 8
WS = 40          # output W columns per core
HALO = 4
WL = WS + 2 * HALO   # 48 local columns with halo

# resolutions exactly as reference._resolutions() computes them (float64
# log/exp path, margins ~1e-6 -- safe to hardcode)
RES = np.array([[16, 16, 4], [24, 22, 5], [37, 31, 8], [57, 45, 11],
                [88, 63, 16], [135, 90, 24], [208, 127, 34], [320, 179, 50]],
               dtype=np.int32)
PRIMES = np.array([1, 2654435761, 805459861], dtype=np.uint32)

NY = [int(RES[l][1]) + 1 for l in range(L)]
NT = [int(RES[l][2]) + 1 for l in range(L)]

# per-core local table x-extent. For core with first local column at global
# cx0: events at local column lw have global ix = floor(x*rx) with
# x*320 in [cx0+lw-0.5, cx0+lw+0.5] =>
#   ix in [floor((2*(cx0+lw)-1)*rx/640), floor((2*(cx0+lw)+1)*rx/640)]
# (exact integer arithmetic; +-1 f32 safety added except rx=320 where the
# relation is exact because the same f32 product feeds round and floor).

def _ix_min(cxg, l):
    rx = int(RES[l][0])
    v = ((2 * cxg - 1) * rx) // 640
    return v if rx == 320 else v - 1


def _ix_max(cxg, l):
    rx = int(RES[l][0])
    v = ((2 * cxg + 1) * rx) // 640
    return v if rx == 320 else v + 1


def _ixbase(core, l):
    cx0 = WS * core - HALO
    return _ix_min(cx0, l)


IXW = []
for l in range(L):
    w = 0
    for core in range(NCORES):
        cx0 = WS * core - HALO
        w = max(w, _ix_max(cx0 + WL - 1, l) + 1 - _ixbase(core, l) + 1)
    IXW.append(w)
LBASE = np.concatenate([[0], np.cumsum([IXW[l] * NY[l] * NT[l] for l in range(L)])]).astype(np.int64)
TOTROW = int(LBASE[-1])
EST = 64  # elements per expanded-table row (32 used + pad to 256B)

# token geometry: 240-cell half-buckets; image-edge columns (cx=0/319, i.e.
# local lw=4 on core 0 and lw=47 on core 7) catch 1.5x density from clipping,
# so buckets 3 and 35 get extra capacity.
HB = 240
NHB = (WL * H) // HB            # 36 half-buckets per frame
PAD_CELLREL = 10000.0
BANDR = 32768  # int16 index reach
SLABMAX = HB      # one-hot width (240)

# slab entries: one 240-cell bucket each (nb=2 slabs fail HW execution
# on this runtime -- redacted INTERNAL error; keep nb=1)
SLABS = [(k, 1) for k in range(NHB)]

# gather calls per slab entry: levels 0..6 whole entry; level 7 per bucket
def _entry_calls(k0, nb):
    calls = [(l, -1) for l in range(7)]
    for hb in range(nb):
        calls.append((7, hb) if nb > 1 else (7, -1))
    return calls


NQ = 4  # SWDGE queues (ucode max); gathers round-robin across them


def _ncalls():
    return B * sum(len(_entry_calls(k0, nb)) for (k0, nb) in SLABS)


def _set_caps(tcap):
    """(Re)derive the token geometry from per-bucket tile capacities."""
    global TCAP, HBOFF, TILESF, TOKF, NTOK, TPSMAX, IDXCOLS_ALL
    TCAP = list(tcap)
    HBOFF = np.concatenate([[0], np.cumsum(TCAP)]).astype(np.int64)
    TILESF = int(HBOFF[-1])
    TOKF = TILESF * 128
    NTOK = B * TOKF
    TPSMAX = max(sum(TCAP[k0:k0 + nb]) for (k0, nb) in SLABS)
    IDXCOLS_ALL = B * sum(
        (sum(TCAP[k0:k0 + nb]) * 8) * 7 + sum(TCAP[k0 + hb] * 8 for hb in range(nb))
        for (k0, nb) in SLABS)


_set_caps([8 if k in (3, 35) else 6 for k in range(NHB)])

# conv strip geometry. Strip s computes output rows cy in [45s, 45s+45)
# at window t = cy - cy0 with cy0 = 45s - 4. Each conv layer shrinks the
# computed window by one row per side (validity cascade); windows are
# clamped to the image so out-of-image rows stay memset-zero (= conv
# 'SAME' padding), and lengths forced even (fp32r matmul ISA rule). When
# parity needs fixing: if the bottom is image-clamped, shrink the top
# (the lost top row only feeds outputs that are discarded); else extend
# down one junk row that no later layer reads.
NS = 4
SROWS = H // NS                  # 45 output rows per strip
TT = SROWS + 8                   # 53 rows per strip window
CY0 = [SROWS * s - 4 for s in range(NS)]
MLO = 4                          # output rows are t in [4, 49)


def _strip_windows(cy0):
    lo1 = max(1, -cy0)
    hi1 = min(TT, H - cy0)
    if (hi1 - lo1) % 2:
        if -cy0 >= 1:
            hi1 -= 1
        else:
            lo1 -= 1
    ws = {1: (lo1, hi1)}
    for k, (bl, bh) in ((2, (2, 52)), (3, (3, 51)), (4, (4, 50))):
        lo = max(bl, -cy0)
        hi = min(bh, H - cy0)
        if (hi - lo) % 2:
            if -cy0 >= bl:
                hi -= 1
            else:
                lo -= 1
        ws[k] = (lo, hi)
    assert ws[4][1] - ws[4][0] == SROWS + 1, (cy0, ws)
    return ws


NLW = 50                         # lw blocks incl +-1 pads (real lw at block lw+1)
LWQ = 17                         # ceil(50/3) : x0 packed [f+32*(lwb%3), (lwb//3)*182+u]
X0C = LWQ * (H + 2) + 2          # 3094 cols + 2 pad
CPSW = 6 * 52                    # conv psum tile width (max 6 lws x 52 rows)

OUTPX = B * WS * H               # 14400


def _band_from_lws(l, lwlo, lwhi):
    """core-independent band start: min over cores of the local ix lower
    bound for columns [lwlo, lwhi]; asserts the worst-case width fits."""
    lo = min(_ix_min(WS * core - HALO + lwlo, l) - _ixbase(core, l)
             for core in range(NCORES))
    hi = max(_ix_max(WS * core - HALO + lwhi, l) - _ixbase(core, l)
             for core in range(NCORES))
    lo = max(0, lo)
    assert (hi - lo + 1) * NY[l] * NT[l] <= BANDR, (l, lwlo, lwhi, lo, hi)
    start = lo * NY[l] * NT[l]
    maxstart = max(0, IXW[l] * NY[l] * NT[l] - BANDR)
    return min(start, maxstart)


def _cells_band(l, c0, c1):
    return _band_from_lws(l, c0 // H, c1 // H)


# ---------------------------------------------------------------- host prep

def _expand_tables(hash_tables):
    """Build the per-core-family expanded tables. Returns list of 8 arrays
    [TOTROW, EST] float32 (one per core; they differ by x-slice)."""
    ht = np.asarray(hash_tables, dtype=np.float32)  # (L, TBL, F)
    out = []
    for core in range(NCORES):
        exp = np.zeros((TOTROW, EST), dtype=np.float32)
        cx0 = WS * core - HALO  # global cx of local lw=0
        for l in range(L):
            rx = int(RES[l][0])
            ny, nt = NY[l], NT[l]
            ixbase = _ixbase(core, l)
            ixs = ixbase + np.arange(IXW[l])              # global ix values
            iys = np.arange(ny)
            its = np.arange(nt)
            IX, IY, IT = np.meshgrid(ixs, iys, its, indexing="ij")
            rows = np.zeros((IXW[l], ny, nt, F, 8), dtype=np.float32)
            for c in range(8):
                ox, oy, ot = c & 1, (c >> 1) & 1, (c >> 2) & 1
                a = (IX + ox).astype(np.int64)
                # clip negatives/overflow: events never index there, value irrelevant
                a = np.clip(a, 0, None).astype(np.uint32)
                b = (IY + oy).astype(np.uint32)
                d = (IT + ot).astype(np.uint32)
                hsh = ((a * PRIMES[0]) ^ (b * PRIMES[1]) ^ (d * PRIMES[2])) % TBL
                rows[:, :, :, :, c] = ht[l][hsh.astype(np.int64)]
            exp[LBASE[l]:LBASE[l + 1], :32] = rows.reshape(-1, 32)
        out.append(exp)
    return out


def _prep_events(currentBlock, eventCounts):
    """Shard + sort + slab-bucket events. Returns per-core dicts with
    coords [128, 4*NTOK/128] f32 and idx16 [128, IDXCOLS_ALL] i16,
    or None if any slab overflows its fixed capacity."""
    cb = np.asarray(currentBlock, dtype=np.float32)
    ec = np.asarray(eventCounts).astype(np.int64)
    n = cb.shape[0]
    bidx = np.repeat(np.arange(B, dtype=np.int64), ec, )
    if bidx.shape[0] != n:
        # pad/truncate defensively (counts should sum to N)
        bidx = np.resize(bidx, n)
    x, y = cb[:, 0], cb[:, 1]
    cx = np.clip(np.round(x * np.float32(W)), 0, W - 1).astype(np.int64)
    cy = np.clip(np.round(y * np.float32(H)), 0, H - 1).astype(np.int64)

    # per (event, level) local cell ids (host-side integer planning)
    scaled = cb[:, None, :] * RES[None, :, :].astype(np.float32)   # (n, L, 3)
    base = np.floor(scaled).astype(np.int64)                        # (n, L, 3)

    cores = []
    for core in range(NCORES):
        cx0 = WS * core - HALO
        sel = (cx >= cx0) & (cx < cx0 + WL)
        ei = np.nonzero(sel)[0]
        lw = cx[ei] - cx0
        cell = lw * H + cy[ei]
        fr = bidx[ei]
        order = np.lexsort((cell, fr))
        ei, lw, cell, fr = ei[order], lw[order], cell[order], fr[order]

        coords = np.full((128, (NTOK // 128) * 4), 0.5, dtype=np.float32)
        coords[:, 3::4] = PAD_CELLREL
        # -1 = "skip" (trailing negative idxs generate no DMA descriptors);
        # each call's valid prefix is padded to a 16-multiple with idx 0.
        idx16 = np.full((16, IDXCOLS_ALL), -1, dtype=np.int16)
        gcnt = np.zeros(_ncalls(), dtype=np.int32)
        gidx = 0

        ok = True
        for f in range(B):
            m = fr == f
            ce = cell[m]
            eidx = ei[m]
            hb_of = ce // HB
            cc = f * (IDXCOLS_ALL // B)
            for (k0, nb) in SLABS:
                # place events of each bucket into its fixed tile range
                toks_l, evs_l, cells_l = [], [], []
                ent_tok0 = f * TOKF + int(HBOFF[k0]) * 128
                rel0 = 0
                for hb in range(nb):
                    selh = hb_of == k0 + hb
                    cnt = int(selh.sum())
                    if cnt > TCAP[k0 + hb] * 128:
                        ok = False
                        break
                    toks_l.append(ent_tok0 + rel0 + np.arange(cnt))
                    evs_l.append(eidx[selh])
                    cells_l.append(ce[selh])
                    rel0 += TCAP[k0 + hb] * 128
                if not ok:
                    break
                toks = np.concatenate(toks_l)
                evs = np.concatenate(evs_l)
                cells_s = np.concatenate(cells_l)
                tp, tc = toks % 128, toks // 128
                coords[tp, tc * 4 + 0] = cb[evs, 0]
                coords[tp, tc * 4 + 1] = cb[evs, 1]
                coords[tp, tc * 4 + 2] = cb[evs, 2]
                coords[tp, tc * 4 + 3] = (cells_s - k0 * HB).astype(np.float32)
                # gather indices per call
                trel_all = toks - ent_tok0
                ent_c0 = k0 * HB
                ent_c1 = (k0 + nb) * HB - 1
                for (l, h) in _entry_calls(k0, nb):
                    ny, nt = NY[l], NT[l]
                    if h == -1:
                        tlo = 0
                        thi = sum(TCAP[k0:k0 + nb]) * 128
                        band = _cells_band(l, ent_c0, ent_c1)
                        ncol = (thi - tlo) // 16
                    else:
                        tlo = sum(TCAP[k0:k0 + h]) * 128
                        thi = tlo + TCAP[k0 + h] * 128
                        band = _cells_band(l, (k0 + h) * HB, (k0 + h + 1) * HB - 1)
                        ncol = (thi - tlo) // 16
                    inr = (trel_all >= tlo) & (trel_all < thi)
                    cntc = int(inr.sum())
                    if cntc:
                        evr = evs[inr]
                        bx = base[evr, l, 0] - _ixbase(core, l)
                        by = base[evr, l, 1]
                        bt = base[evr, l, 2]
                        loc = (bx * ny + by) * nt + bt - band
                        if loc.min() < 0 or loc.max() >= BANDR:
                            ok = False
                        trel = trel_all[inr] - tlo
                        idx16[trel % 16, cc + trel // 16] = loc.astype(np.int16)
                    nvalid = max(16, -(-cntc // 16) * 16)
                    if nvalid > cntc:
                        tp_ = np.arange(cntc, nvalid)
                        idx16[tp_ % 16, cc + tp_ // 16] = 0
                    gcnt[gidx] = nvalid
                    gidx += 1
                    cc += ncol
                if not ok:
                    break
            if not ok:
                break
        if not ok:
            return None
        cores.append({"coords": coords,
                      "idx16": np.tile(idx16, (8, 1)),
                      "gcnt": np.tile(gcnt[None, :], (16, 1))})
    return cores


def _repack_weights(inp):
    """conv/MLP weights into lhsT layouts."""
    w1 = np.asarray(inp["conv1_w"], np.float32)   # (256, 32, 3, 3)
    w2 = np.asarray(inp["conv2_w"], np.float32)   # (256, 256, 3, 3)
    w3 = np.asarray(inp["conv3_w"], np.float32)
    w4 = np.asarray(inp["conv4_w"], np.float32)   # (128, 256, 3, 3)

    # conv1 lhsT packed K=96: row g*32+c = input channel c at kernel row
    # dh=g (x0 is stored as 3 dh-shifted partition copies), col block (dw,oc)
    km1 = np.zeros((96, 6 * 128), np.float32)
    for dw in range(3):
        for g in range(3):
            for oc in range(2):
                km1[g * 32:(g + 1) * 32, (dw * 2 + oc) * 128:(dw * 2 + oc + 1) * 128] = \
                    w1[oc * 128:(oc + 1) * 128, :, dw, g].T
    import ml_dtypes
    km1 = km1.astype(ml_dtypes.bfloat16)
    def pack_big(w, nco):
        # -> [nco, 128, 18*128]: [oc][i][(ab*2+kc)*128+o]
        out = np.zeros((nco, 128, 18 * 128), np.float32)
        for a in range(3):
            for b_ in range(3):
                ab = a * 3 + b_
                for kc in range(2):
                    for oc in range(nco):
                        out[oc, :, (ab * 2 + kc) * 128:(ab * 2 + kc + 1) * 128] = \
                            w[oc * 128:(oc + 1) * 128, kc * 128:(kc + 1) * 128, a, b_].T
        return out
    km2 = pack_big(w2, 2)
    km3 = pack_big(w3, 2)
    km4 = pack_big(w4, 1)[0]      # [128, 2304]

    return {
        "km1": km1, "km2": km2, "km3": km3, "km4": km4,
        "cb1": np.asarray(inp["conv1_b"], np.float32),
        "cb2": np.asarray(inp["conv2_b"], np.float32),
        "cb3": np.asarray(inp["conv3_b"], np.float32),
        "cb4": np.asarray(inp["conv4_b"], np.float32),
        "mw0": np.asarray(inp["mlp0_w"], np.float32),
        "mw1": np.asarray(inp["mlp1_w"], np.float32),
        "mw2": np.asarray(inp["mlp2_w"], np.float32),
        "mw3": np.asarray(inp["mlp3_w"], np.float32),
        "mb0": np.asarray(inp["mlp0_b"], np.float32),
        "mb1": np.asarray(inp["mlp1_b"], np.float32),
        "mb2": np.asarray(inp["mlp2_b"], np.float32),
        "mb3": np.asarray(inp["mlp3_b"], np.float32),
    }


# ---------------------------------------------------------------- walrus fix

def _fix_walrus_wait_limit(nc):
    """This walrus build rejects >1 sem wait on most instructions. Hoist
    extra waits onto same-engine NoOp carriers (EventSemaphore excluded:
    its waits are event monitors, not engine-blocking)."""
    import concourse.mybir as mybir
    n_fixed = 0
    for fn in nc.m.functions:
        for bb in fn.blocks:
            changed = False
            new_insts = []
            for inst in bb.instructions:
                si = inst.sync_info
                if type(inst).__name__ == "InstEventSemaphore":
                    new_insts.append(inst)
                    continue
                if si is not None and len(si.on_wait) > 1:
                    waits = list(si.on_wait)
                    for w in waits[:-1]:
                        eng = nc.engines[inst.engine]
                        bi = eng.nop()
                        carrier = bi.ins
                        cur = nc.cur_bb.bb
                        lst = cur.instructions
                        assert lst and lst[-1].name == carrier.name
                        cur.instructions = lst[:-1]
                        carrier.sync_info = mybir.SyncInfo(on_wait=[w], on_update=[])
                        new_insts.append(carrier)
                    si.on_wait = waits[-1:]
                    changed = True
                    n_fixed += 1
                new_insts.append(inst)
            if changed:
                bb.instructions = new_insts
    return n_fixed


# ---------------------------------------------------------------- device IR

_PROG_CACHE = {}


def _build_program():
    key = tuple(TCAP)
    if key in _PROG_CACHE:
        return _PROG_CACHE[key]
    import concourse.bass as bass
    import concourse.bacc as bacc
    import concourse.mybir as mybir
    import concourse.tile as tile

    f32, i16, i32 = mybir.dt.float32, mybir.dt.int16, mybir.dt.int32
    bf16 = mybir.dt.bfloat16
    f32r = mybir.dt.float32r
    AF = mybir.ActivationFunctionType
    OP = mybir.AluOpType

    nc = bacc.Bacc("TRN2", target_bir_lowering=False, debug=False,
                   num_swdge_queues=NQ)

    coords = nc.declare_dram_parameter("coords", [128, (NTOK // 128) * 4], f32, isOutput=False)
    idx16 = nc.declare_dram_parameter("idx16", [128, IDXCOLS_ALL], i16, isOutput=False)
    gcntd = nc.declare_dram_parameter("gcnt", [16, _ncalls()], i32, isOutput=False)
    expt = nc.declare_dram_parameter("expt", [TOTROW, EST], f32, isOutput=False)
    resc = nc.declare_dram_parameter("resc", [128, 24], f32, isOutput=False)
    emaskd = nc.declare_dram_parameter("emask", [128, 8], f32, isOutput=False)
    iotad = nc.declare_dram_parameter("iotad", [128, SLABMAX], f32, isOutput=False)
    km1d = nc.declare_dram_parameter("km1", [96, 768], bf16, isOutput=False)
    km2d = nc.declare_dram_parameter("km2", [2, 128, 2304], f32r, isOutput=False)
    km3d = nc.declare_dram_parameter("km3", [2, 128, 2304], f32r, isOutput=False)
    km4d = nc.declare_dram_parameter("km4", [128, 2304], f32r, isOutput=False)
    cb1d = nc.declare_dram_parameter("cb1", [256], f32, isOutput=False)
    cb2d = nc.declare_dram_parameter("cb2", [256], f32, isOutput=False)
    cb3d = nc.declare_dram_parameter("cb3", [256], f32, isOutput=False)
    cb4d = nc.declare_dram_parameter("cb4", [128], f32, isOutput=False)
    mw0d = nc.declare_dram_parameter("mw0", [128, 128], f32r, isOutput=False)
    mw1d = nc.declare_dram_parameter("mw1", [128, 64], f32r, isOutput=False)
    mw2d = nc.declare_dram_parameter("mw2", [64, 32], f32r, isOutput=False)
    mw3d = nc.declare_dram_parameter("mw3", [32, 20], f32r, isOutput=False)
    mb0d = nc.declare_dram_parameter("mb0", [128], f32, isOutput=False)
    mb1d = nc.declare_dram_parameter("mb1", [64], f32, isOutput=False)
    mb2d = nc.declare_dram_parameter("mb2", [32], f32, isOutput=False)
    mb3d = nc.declare_dram_parameter("mb3", [20], f32, isOutput=False)
    outd = nc.declare_dram_parameter("out", [20, OUTPX], f32, isOutput=True)
    DBG = bool(int(__import__("os").environ.get("KDBG", "0")))
    if DBG:
        dbgd = nc.declare_dram_parameter("dbg_x0", [32, 3 * X0C], f32, isOutput=True)
        dbgy = nc.declare_dram_parameter("dbg_y", [128, 2 * NLW * TT], f32, isOutput=True)
        dbgy4 = nc.declare_dram_parameter("dbg_y4", [128, 2 * NLW * TT], f32, isOutput=True)

    def fap(tap, off, dims):
        return bass.AP(tap.tensor, tap.offset + off, [list(tap.ap[0])] + [list(d) for d in dims])

    def pslice(tap, p0, pn, off, dims):
        p = list(tap.ap[0])
        newp = [p[0], pn]
        return bass.AP(tap.tensor, tap.offset + p0 * p[0] + off, [newp] + [list(d) for d in dims])

    with tile.TileContext(nc) as tc:
        with nc.allow_low_precision(reason="fp32r matmul operands (same fp32 bits)"), \
             tc.tile_pool(name="const", bufs=1) as cp, \
             tc.tile_pool(name="wstream", bufs=2) as wsp, \
             tc.tile_pool(name="idxp", bufs=3) as ixp, \
             tc.tile_pool(name="vt", bufs=3) as vtp, \
             tc.tile_pool(name="enc", bufs=1) as ep, \
             tc.tile_pool(name="sp", bufs=3) as sp_, \
             tc.tile_pool(name="x0p", bufs=2) as xp, \
             tc.tile_pool(name="yp", bufs=2) as yp, \
             tc.tile_pool(name="ost", bufs=2) as osp, \
             tc.tile_pool(name="cpsum", bufs=3, space="PSUM") as cps, \
             tc.tile_pool(name="spsum", bufs=2, space="PSUM") as sps, \
             tc.tile_pool(name="mpsum", bufs=2, space="PSUM") as mps:

            # ---------------- constants
            coords_t = cp.tile([128, (NTOK // 128) * 4], f32)
            nc.sync.dma_start(out=coords_t[:], in_=coords[:])
            gcnt_t = cp.tile([16, _ncalls()], i32)
            nc.sync.dma_start(out=gcnt_t[:], in_=gcntd[:])
            resc_t = cp.tile([128, 24], f32)
            nc.sync.dma_start(out=resc_t[:], in_=resc[:])
            emask_t = cp.tile([128, 8], f32)
            nc.sync.dma_start(out=emask_t[:], in_=emaskd[:])
            iota_t = cp.tile([128, SLABMAX], f32)
            nc.sync.dma_start(out=iota_t[:], in_=iotad[:])
            zero_t = cp.tile([128, 1], f32)
            nc.vector.memset(zero_t[:], 0.0)
            one_t = cp.tile([128, 1], f32)
            nc.vector.memset(one_t[:], 1.0)
            km1_t = cp.tile([96, 768], bf16)
            nc.sync.dma_start(out=km1_t[:], in_=km1d[:])
            km4_t = cp.tile([128, 2304], f32r)
            nc.sync.dma_start(out=km4_t[:], in_=km4d[:])
            mw0_t = cp.tile([128, 128], f32r)
            nc.sync.dma_start(out=mw0_t[:], in_=mw0d[:])
            mw1_t = cp.tile([128, 64], f32r)
            nc.sync.dma_start(out=mw1_t[:], in_=mw1d[:])
            mw2_t = cp.tile([64, 32], f32r)
            nc.sync.dma_start(out=mw2_t[:], in_=mw2d[:])
            mw3_t = cp.tile([32, 20], f32r)
            nc.sync.dma_start(out=mw3_t[:], in_=mw3d[:])

            def bias_tile(dram, o0, n):
                t = cp.tile([n, 1], f32, tag=f"bias{dram.name}{o0}")
                nc.sync.dma_start(out=t[:], in_=dram[o0:o0 + n, None])
                return t
            cb1_t = [bias_tile(cb1d, o * 128, 128) for o in range(2)]
            cb2_t = [bias_tile(cb2d, o * 128, 128) for o in range(2)]
            cb3_t = [bias_tile(cb3d, o * 128, 128) for o in range(2)]
            cb4_t = [bias_tile(cb4d, 0, 128)]
            mb0_t = bias_tile(mb0d, 0, 128)
            mb1_t = bias_tile(mb1d, 0, 64)
            mb2_t = bias_tile(mb2d, 0, 32)
            mb3_t = bias_tile(mb3d, 0, 20)

            x0_tiles = {}

            # prime the vt ring: skipped (-1) gather slots keep stale SBUF
            # data, which must be finite (pad tokens have nonzero trilinear
            # weights; their one-hot scatter row is all-zero, so any finite
            # value is harmless -- NaN is not).
            for _ in range(3):
                vt0 = vtp.tile([128, L * TPSMAX * EST], f32, tag="vt")
                nc.vector.memset(vt0[:], 0.0)

            gcall = [0]
            # one shared Pool register for the per-call gather counts (a
            # fresh value_load per call exhausts Pool's register file; Pool
            # executes in order so reuse is WAR-safe)
            gcnt_reg = nc.gpsimd.alloc_register("gcnt_reg")

            # ---------------- encode one frame
            def encode_frame(f):
                x0_t = xp.tile([96, 3 * X0C], bf16, tag="x0")
                nc.vector.memset(x0_t[:], 0.0)
                x0_tiles[f] = x0_t
                cc_base = f * (IDXCOLS_ALL // B)
                for (k0, nb) in SLABS:
                    tps = sum(TCAP[k0:k0 + nb])       # tiles in this entry (<=12)
                    ncells = nb * HB
                    tcol0 = f * TILESF + int(HBOFF[k0])
                    ncols_entry = tps * 8 * 7 + tps * 8
                    ix_t = ixp.tile([128, TPSMAX * 8 * 8], i16, tag="ix")
                    nc.sync.dma_start(
                        out=ix_t[:, :ncols_entry],
                        in_=idx16[:, cc_base:cc_base + ncols_entry])
                    cc_base += ncols_entry
                    vt = vtp.tile([128, L * TPSMAX * EST], f32, tag="vt")
                    cc = 0
                    # all calls of an nb=1 entry share the same valid count:
                    # one register load serves the whole entry
                    assert nb == 1
                    nc.gpsimd.reg_load(gcnt_reg, gcnt_t[0:1, gcall[0]:gcall[0] + 1])
                    for (l, h) in _entry_calls(k0, nb):
                        if h == -1:
                            band = _cells_band(l, k0 * HB, (k0 + nb) * HB - 1)
                            ntok_c = tps * 128
                            vt_off = l * tps * EST
                        else:
                            band = _cells_band(l, (k0 + h) * HB, (k0 + h + 1) * HB - 1)
                            ntok_c = TCAP[k0 + h] * 128
                            vt_off = 7 * tps * EST + sum(TCAP[k0:k0 + h]) * EST
                        rstart = int(LBASE[l]) + band
                        nrow = min(BANDR, int(LBASE[l + 1]) - rstart)
                        gi = gcall[0]
                        gcall[0] += 1
                        nc.gpsimd.dma_gather(
                            out_ap=fap(vt[:], vt_off, [[EST, ntok_c // 128], [1, EST]]),
                            in_ap=expt[rstart:rstart + nrow, :],
                            idxs_ap=ix_t[:, cc:cc + ntok_c // 16],
                            num_idxs=ntok_c, num_idxs_reg=gcnt_reg,
                            elem_size=EST, single_packet=False,
                            queue_num=gi % NQ)
                        cc += ntok_c // 16

                    n3 = L * tps * 3
                    scaled = ep.tile([128, L * TPSMAX * 3], f32, tag="scaled")
                    nc.vector.tensor_tensor(
                        out=scaled[:, :n3],
                        in0=fap(coords_t[:], tcol0 * 4, [[0, L], [4, tps], [1, 3]]),
                        in1=fap(resc_t[:], 0, [[3, L], [0, tps], [1, 3]]),
                        op=OP.mult)
                    ci = ep.tile([128, L * TPSMAX * 3], i32, tag="ci")
                    nc.vector.tensor_copy(out=ci[:, :n3], in_=scaled[:, :n3])
                    cf = ep.tile([128, L * TPSMAX * 3], f32, tag="cf")
                    nc.vector.tensor_copy(out=cf[:, :n3], in_=ci[:, :n3])
                    f0 = ep.tile([128, L * TPSMAX * 3], f32, tag="f0")
                    nc.vector.tensor_tensor(out=f0[:, :n3], in0=scaled[:, :n3], in1=cf[:, :n3], op=OP.subtract)
                    neg = ep.tile([128, L * TPSMAX * 3], f32, tag="neg")
                    nc.vector.tensor_tensor(out=neg[:, :n3], in0=f0[:, :n3], in1=zero_t[:].to_broadcast([128, n3]), op=OP.is_lt)
                    frac = ep.tile([128, L * TPSMAX * 3], f32, tag="frac")
                    nc.vector.tensor_tensor(out=frac[:, :n3], in0=f0[:, :n3], in1=neg[:, :n3], op=OP.add)
                    # F2 = (1-frac, frac) interleaved  (l, g, d, 2)
                    F2 = ep.tile([128, L * TPSMAX * 6], f32, tag="F2")
                    F2S = [[6 * tps, L], [6, tps], [2, 3]]
                    SH = [[3 * tps, L], [3, tps], [1, 3]]
                    nc.vector.tensor_tensor(
                        out=fap(F2[:], 0, F2S),
                        in0=one_t[:].to_broadcast([128, L, tps, 3]),
                        in1=fap(frac[:], 0, SH), op=OP.subtract)
                    nc.vector.tensor_copy(out=fap(F2[:], 1, F2S), in_=fap(frac[:], 0, SH))
                    # wxy (l, g, cy, cx)
                    wxy = ep.tile([128, L * TPSMAX * 4], f32, tag="wxy")
                    nc.vector.tensor_tensor(
                        out=wxy[:, :L * tps * 4],
                        in0=fap(F2[:], 0, [[6 * tps, L], [6, tps], [0, 2], [1, 2]]),
                        in1=fap(F2[:], 2, [[6 * tps, L], [6, tps], [1, 2], [0, 2]]),
                        op=OP.mult)
                    # w8 (l, g, ct, cy, cx)
                    w8 = ep.tile([128, L * TPSMAX * 8], f32, tag="w8")
                    nc.vector.tensor_tensor(
                        out=w8[:, :L * tps * 8],
                        in0=fap(wxy[:], 0, [[4 * tps, L], [4, tps], [0, 2], [1, 4]]),
                        in1=fap(F2[:], 4, [[6 * tps, L], [6, tps], [1, 2], [0, 4]]),
                        op=OP.mult)
                    # wv = vt * w8   (l, g, f, c)
                    wv = ep.tile([128, L * TPSMAX * 32], f32, tag="wv")
                    nc.vector.tensor_tensor(
                        out=wv[:, :L * tps * 32],
                        in0=fap(vt[:], 0, [[tps * EST, L], [EST, tps], [8, F], [1, 8]]),
                        in1=fap(w8[:], 0, [[8 * tps, L], [8, tps], [0, F], [1, 8]]),
                        op=OP.mult)
                    # encp in tile-major (g, l, f) order so the per-tile
                    # matmul lhsT slice is one contiguous 32-wide free dim
                    encp = ep.tile([128, L * TPSMAX * 4], f32, tag="encp")
                    with nc.allow_low_precision(reason="f32r matmul operand (fp32 bits)"):
                        nc.vector.tensor_reduce(
                            out=fap(encp[:], 0, [[4, L], [32, tps], [1, 4]]).bitcast(f32r),
                            in_=wv[:, :L * tps * 32].rearrange("p (a c) -> p a c", c=8),
                            op=OP.add, axis=mybir.AxisListType.X)
                    # one-hot scatter matmuls into the slab psum
                    ps = sps.tile([32, SLABMAX], f32, tag="sps")
                    for tt in range(tps):
                        S_t = sp_.tile([128, SLABMAX], f32r, tag="S")
                        nc.vector.tensor_tensor(
                            out=S_t[:, :ncells].bitcast(f32r),
                            in0=fap(coords_t[:], (tcol0 + tt) * 4 + 3, [[0, ncells]]),
                            in1=iota_t[:, :ncells],
                            op=OP.is_equal)
                        nc.tensor.matmul(
                            ps[:, :ncells],
                            lhsT=fap(encp[:], tt * 32, [[1, 32]]).bitcast(f32r),
                            rhs=S_t[:, :ncells].bitcast(f32r),
                            start=(tt == 0), stop=(tt == tps - 1))
                    # evict slab psum -> x0 (lw-run segments)
                    c0s = k0 * HB
                    c1s = c0s + ncells
                    cpos = c0s
                    while cpos < c1s:
                        lw = cpos // H
                        ce = min(c1s, (lw + 1) * H)
                        lwb = lw + 1
                        q, lwq = lwb % 3, lwb // 3
                        u0 = (cpos - lw * H) + 1
                        # write 3 dh-shifted copies into partition groups
                        # 0-31 / 32-63 / 64-95 (copy g holds x0 shifted so a
                        # K=96 conv1 matmul reads all 3 kernel rows at once)
                        for g in range(3):
                            nc.scalar.activation(
                                out=pslice(x0_t[:], 32 * g, 32,
                                           q * X0C + lwq * (H + 2) + u0 + 1 - g,
                                           [[1, ce - cpos]]),
                                in_=ps[:, cpos - c0s:ce - c0s],
                                func=AF.Copy)
                        cpos = ce
                    yield

            # ---------------- conv + mlp one frame
            def mask_edges(yt):
                # zero the out-of-frame lwb columns (core-dependent via the
                # emask input): lwb 1..4 <- emask[0:4], lwb 45..48 <- [4:8]
                for oc in range(2):
                    for (lwb0, m0) in ((1, 0), (45, 4)):
                        ap = fap(yt[:], oc * NLW * TT + lwb0 * TT, [[TT, 4], [1, TT]])
                        nc.vector.tensor_tensor(
                            out=ap, in0=ap,
                            in1=fap(emask_t[:], m0, [[1, 4], [0, TT]]),
                            op=OP.mult)

            KCONV = __import__("os").environ.get("KCONV", "mlp")

            def _dump_dbg(yt):
                # route a slice of the tile to outd so the phase has output
                nc.sync.dma_start(out=outd[:, :2650],
                                  in_=pslice(yt[:], 0, 20, 0, [[1, 2650]]).bitcast(f32))

            def conv_frame(f):
                x0_t = x0_tiles[f]
                for s in range(NS):
                    cy0 = CY0[s]
                    ws = _strip_windows(cy0)
                    t1lo, t1hi = ws[1]
                    y1 = yp.tile([128, 2 * NLW * TT], f32r, tag="y")
                    nc.vector.memset(y1[:].bitcast(f32), 0.0)
                    # conv1: K=96 (3 dh rows baked into x0's partition copies)
                    for oc in range(2):
                        for q in range(3):
                            for half in range(3):
                                lws = [lwb for lwb in range(1, 49) if lwb % 3 == q][half * 6:(half + 1) * 6]
                                if not lws:
                                    continue
                                n_lw = len(lws)
                                ps = cps.tile([128, CPSW], f32, tag="cps")
                                nlen = t1hi - t1lo
                                for dw in range(3):
                                    qin = (lws[0] + dw - 1) % 3
                                    lwqin = (lws[0] + dw - 1) // 3
                                    rhs = fap(x0_t[:],
                                              qin * X0C + lwqin * (H + 2) + cy0 + t1lo + 1,
                                              [[H + 2, n_lw], [1, nlen]])
                                    nc.tensor.matmul(
                                        ps[:, :n_lw * nlen],
                                        lhsT=km1_t[:, (dw * 2 + oc) * 128:(dw * 2 + oc + 1) * 128],
                                        rhs=rhs,
                                        start=(dw == 0), stop=(dw == 2))
                                # evict with relu+bias into y1 (strided by 4 lw)
                                nc.scalar.activation(
                                    out=fap(y1[:], oc * NLW * TT + lws[0] * TT + t1lo,
                                            [[3 * TT, n_lw], [1, nlen]]),
                                    in_=fap(ps[:], 0, [[nlen, n_lw], [1, nlen]]),
                                    func=AF.Relu, bias=cb1_t[oc][:])
                    mask_edges(y1)
                    yield
                    if KCONV == "c1":
                        if f == 0 and s == 0:
                            _dump_dbg(y1)
                        continue
                    # conv2 / conv3
                    prev = y1
                    for layer, (kmd, cbt) in enumerate([(km2d, cb2_t), (km3d, cb3_t)]):
                        lo, hi = ws[2 + layer]
                        nlen = hi - lo
                        ynext = yp.tile([128, 2 * NLW * TT], f32r, tag="y")
                        nc.vector.memset(ynext[:].bitcast(f32), 0.0)
                        for oc in range(2):
                            wt = wsp.tile([128, 2304], f32r, tag="w")
                            nc.scalar.dma_start(out=wt[:], in_=kmd[oc])
                            for g in range(8):
                                lw0 = 1 + 6 * g
                                n_lw = 6
                                ps = cps.tile([128, CPSW], f32, tag="cps")
                                idx = 0
                                for ab in range(9):
                                    dw, dh = ab // 3, ab % 3
                                    for kc in range(2):
                                        rhs = fap(prev[:], kc * NLW * TT + (lw0 + dw - 1) * TT + lo + dh - 1,
                                                  [[TT, n_lw], [1, nlen]])
                                        nc.tensor.matmul(
                                            ps[:, :n_lw * nlen],
                                            lhsT=wt[:, (ab * 2 + kc) * 128:(ab * 2 + kc + 1) * 128].bitcast(f32r),
                                            rhs=rhs.bitcast(f32r),
                                            start=(idx == 0), stop=(idx == 17))
                                        idx += 1
                                nc.scalar.activation(
                                    out=fap(ynext[:], oc * NLW * TT + lw0 * TT + lo,
                                            [[TT, n_lw], [1, nlen]]),
                                    in_=fap(ps[:], 0, [[nlen, n_lw], [1, nlen]]),
                                    func=AF.Relu, bias=cbt[oc][:])
                        mask_edges(ynext)
                        prev = ynext
                        yield
                        if KCONV == "c2" and layer == 0:
                            break
                        if DBG and f == 0 and s == 0 and layer == 0:
                            nc.sync.dma_start(out=dbgy[:], in_=ynext[:].bitcast(f32))
                    if KCONV in ("c2", "c3"):
                        if f == 0 and s == 0:
                            _dump_dbg(prev)
                        continue
                    # conv4 (oc=1)
                    lo4, hi4 = ws[4]
                    nlen = hi4 - lo4
                    y4 = yp.tile([128, 2 * NLW * TT], f32r, tag="y")
                    if DBG:
                        nc.vector.memset(y4[:].bitcast(f32), 0.0)
                    for g in range(8):
                        lw0 = 1 + 6 * g
                        n_lw = 6
                        ps = cps.tile([128, CPSW], f32, tag="cps")
                        idx = 0
                        for ab in range(9):
                            dw, dh = ab // 3, ab % 3
                            for kc in range(2):
                                rhs = fap(prev[:], kc * NLW * TT + (lw0 + dw - 1) * TT + lo4 + dh - 1,
                                          [[TT, n_lw], [1, nlen]])
                                nc.tensor.matmul(
                                    ps[:, :n_lw * nlen],
                                    lhsT=km4_t[:, (ab * 2 + kc) * 128:(ab * 2 + kc + 1) * 128].bitcast(f32r),
                                    rhs=rhs.bitcast(f32r),
                                    start=(idx == 0), stop=(idx == 17))
                                idx += 1
                        nc.scalar.activation(
                            out=fap(y4[:], lw0 * TT + lo4, [[TT, n_lw], [1, nlen]]),
                            in_=fap(ps[:], 0, [[nlen, n_lw], [1, nlen]]),
                            func=AF.Relu, bias=cb4_t[0][:])
                    yield
                    if DBG and f == 0 and s == 0:
                        nc.sync.dma_start(out=dbgy4[:], in_=y4[:].bitcast(f32))
                    if KCONV == "c4":
                        if f == 0 and s == 0:
                            _dump_dbg(y4)
                        continue
                    # mlp over y4's 46 computed rows; output rows t in
                    # [MLO, MLO+45) picked out at the DMA (skip = MLO-lo4)
                    MR = SROWS + 1                     # 46, even, 8*46=368>=256
                    skip = MLO - lo4
                    for j in range(5):
                        lw0 = 5 + 8 * j
                        rhs0 = fap(y4[:], lw0 * TT + lo4, [[TT, 8], [1, MR]])
                        p0 = mps.tile([128, 8 * MR], f32, tag="mp")
                        nc.tensor.matmul(p0[:], lhsT=mw0_t[:].bitcast(f32r), rhs=rhs0.bitcast(f32r), start=True, stop=True)
                        h1 = osp.tile([128, 8 * MR], f32r, tag="h1")
                        nc.scalar.activation(out=h1[:], in_=p0[:], func=AF.Relu, bias=mb0_t[:])
                        p1 = mps.tile([64, 8 * MR], f32, tag="mp")
                        nc.tensor.matmul(p1[:], lhsT=mw1_t[:].bitcast(f32r), rhs=h1[:].bitcast(f32r), start=True, stop=True)
                        h2 = osp.tile([64, 8 * MR], f32r, tag="h2")
                        nc.scalar.activation(out=h2[:], in_=p1[:], func=AF.Relu, bias=mb1_t[:])
                        p2 = mps.tile([32, 8 * MR], f32, tag="mp")
                        nc.tensor.matmul(p2[:], lhsT=mw2_t[:].bitcast(f32r), rhs=h2[:].bitcast(f32r), start=True, stop=True)
                        h3 = osp.tile([32, 8 * MR], f32r, tag="h3")
                        nc.scalar.activation(out=h3[:], in_=p2[:], func=AF.Relu, bias=mb2_t[:])
                        p3 = mps.tile([20, 8 * MR], f32, tag="mp")
                        nc.tensor.matmul(p3[:], lhsT=mw3_t[:].bitcast(f32r), rhs=h3[:].bitcast(f32r), start=True, stop=True)
                        ho = osp.tile([20, 8 * MR], f32, tag="ho")
                        nc.vector.tensor_tensor(out=ho[:], in0=p3[:], in1=mb3_t[:].to_broadcast([20, 8 * MR]), op=OP.add)
                        nc.scalar.dma_start(
                            out=fap(outd[:], (f * WS + (lw0 - 5)) * H + SROWS * s, [[H, 8], [1, SROWS]]),
                            in_=fap(ho[:], skip, [[MR, 8], [1, SROWS]]))
                    yield

            KPHASE = __import__("os").environ.get("KPHASE", "all")
            def _zero_x0(f):
                t = xp.tile([96, 3 * X0C], bf16, tag="x0")
                nc.vector.memset(t[:], 0.0)
                x0_tiles[f] = t

            if KPHASE == "all":
                for _ in encode_frame(0):
                    pass
                # interleave conv(0) strips with encode(1) entries so the
                # frame-1 gathers run while the PE does the frame-0 convs
                ge = encode_frame(1)
                gc = conv_frame(0)
                nsteps = NS * 5          # conv yield points
                per = -(-len(SLABS) // nsteps)   # ceil: drain encode(1) fully
                                                 # inside the conv(0) window
                while next(gc, "done") != "done":
                    for _ in range(per):
                        if next(ge, "done") == "done":
                            break

                for _ in ge:
                    pass
                for _ in conv_frame(1):
                    pass
            elif KPHASE == "enc":
                for f in range(B):
                    for _ in encode_frame(f):
                        pass
            else:
                for f in range(B):
                    _zero_x0(f)
                    for _ in conv_frame(f):
                        pass
            if DBG:
                dx = cp.tile([32, 3 * X0C], f32, tag="dbgx")
                nc.vector.tensor_copy(out=dx[:], in_=x0_tiles[0][:])
                nc.sync.dma_start(out=dbgd[:], in_=dx[:])
            if KPHASE == "enc":
                zo = cp.tile([20, OUTPX], f32)
                nc.vector.memset(zo[:], 0.0)
                nc.vector.tensor_tensor(
                    out=zo[:20, :96], in0=zo[:20, :96],
                    in1=pslice(x0_tiles[0][:], 0, 20, 0, [[1, 96]]), op=OP.add)
                nc.sync.dma_start(out=outd[:], in_=zo[:])

    nc.compile()
    import os as _os
    if _os.environ.get("NO_WAITFIX", "0") != "1":
        _fix_walrus_wait_limit(nc)
    _PROG_CACHE[key] = nc
    return nc


EDGE_LWB = [1, 2, 3, 4, 45, 46, 47, 48]


def _edge_mask(core):
    """[128, 8] column mask: 0.0 where the edge lwb is out of the frame."""
    cx0 = WS * core - HALO
    m = np.array([1.0 if 0 <= cx0 + (lwb - 1) < W else 0.0 for lwb in EDGE_LWB],
                 dtype=np.float32)
    return np.tile(m[None, :], (128, 1))


# ---------------------------------------------------------------- numpy ref

def _numpy_forward(inp):
    """Fallback faithful forward in numpy (slow)."""
    cb = np.asarray(inp["currentBlock"], np.float32)
    ec = np.asarray(inp["eventCounts"]).astype(np.int64)
    ht = np.asarray(inp["hash_tables"], np.float32)
    n = cb.shape[0]
    scaled = cb[:, None, :] * RES[None].astype(np.float32)
    basef = np.floor(scaled)
    frac = scaled - basef
    base = basef.astype(np.uint32)
    enc = np.zeros((n, L, F), np.float32)
    for c in range(8):
        off = np.array([(c >> d) & 1 for d in range(3)], np.uint32)
        idx = base + off[None, None, :]
        hsh = ((idx[..., 0] * PRIMES[0]) ^ (idx[..., 1] * PRIMES[1]) ^ (idx[..., 2] * PRIMES[2])) % TBL
        w = np.where(off.astype(bool)[None, None, :], frac, 1.0 - frac).prod(-1)
        for l in range(L):
            enc[:, l, :] += w[:, l, None] * ht[l][hsh[:, l].astype(np.int64)]
    enc = enc.reshape(n, L * F)
    bidx = np.repeat(np.arange(B), ec)
    cx = np.clip(np.round(cb[:, 0] * np.float32(W)), 0, W - 1).astype(np.int64)
    cy = np.clip(np.round(cb[:, 1] * np.float32(H)), 0, H - 1).astype(np.int64)
    ff = np.zeros((B, W, H, L * F), np.float32)
    np.add.at(ff, (bidx, cx, cy), enc)
    x = ff.transpose(0, 3, 1, 2)
    def conv(x, w, b):
        Bn, Ci, Wn, Hn = x.shape
        Co = w.shape[0]
        y = np.zeros((Bn, Co, Wn, Hn), np.float32)
        xp = np.pad(x, ((0, 0), (0, 0), (1, 1), (1, 1)))
        for a in range(3):
            for bb in range(3):
                y += np.einsum("oi,biwh->bowh", w[:, :, a, bb],
                               xp[:, :, a:a + Wn, bb:bb + Hn], optimize=True)
        return y + b[None, :, None, None]
    x = np.maximum(conv(x, np.asarray(inp["conv1_w"], np.float32), np.asarray(inp["conv1_b"], np.float32)), 0)
    x = np.maximum(conv(x, np.asarray(inp["conv2_w"], np.float32), np.asarray(inp["conv2_b"], np.float32)), 0)
    x = np.maximum(conv(x, np.asarray(inp["conv3_w"], np.float32), np.asarray(inp["conv3_b"], np.float32)), 0)
    x = np.maximum(conv(x, np.asarray(inp["conv4_w"], np.float32), np.asarray(inp["conv4_b"], np.float32)), 0)
    x = x.transpose(0, 2, 3, 1)
    h1 = np.maximum(x @ np.asarray(inp["mlp0_w"], np.float32) + np.asarray(inp["mlp0_b"], np.float32), 0)
    h2 = np.maximum(h1 @ np.asarray(inp["mlp1_w"], np.float32) + np.asarray(inp["mlp1_b"], np.float32), 0)
    h3 = np.maximum(h2 @ np.asarray(inp["mlp2_w"], np.float32) + np.asarray(inp["mlp2_b"], np.float32), 0)
    return (h3 @ np.asarray(inp["mlp3_w"], np.float32) + np.asarray(inp["mlp3_b"], np.float32)).astype(np.float32)


# ---------------------------------------------------------------- entry

def kernel(**inputs):
    try:
        return _device_kernel(**inputs)
    except Exception as e:  # device/compile failure: stay correct
        print(f"kernel: device path failed ({type(e).__name__}: {e}); numpy fallback")
        return _numpy_forward(inputs)


def _ensure_axon():
    """The caller may have initialized jax on cpu (e.g. to run the jax
    reference). The device run needs the axon backend: reset if needed."""
    import jax
    try:
        devs = jax.devices()
        if len(devs) >= NCORES and "cpu" not in str(devs[0]).lower():
            return
    except Exception:
        pass
    try:
        jax.config.update("jax_platforms", "axon")
    except Exception:
        pass
    try:
        jax.clear_backends()
    except Exception:
        pass
    try:
        from jax._src import xla_bridge as _xb
        _xb._clear_backends()
    except Exception:
        pass
    devs = jax.devices()
    assert len(devs) >= NCORES, f"need {NCORES} devices, got {devs}"


def _fit_caps(currentBlock, eventCounts):
    """Per-bucket tile capacities sized to the actual event distribution
    (max over cores/frames, +1 tile headroom cap at tile granularity)."""
    cb = np.asarray(currentBlock, dtype=np.float32)
    ec = np.asarray(eventCounts).astype(np.int64)
    n = cb.shape[0]
    bidx = np.repeat(np.arange(B, dtype=np.int64), ec)
    if bidx.shape[0] != n:
        bidx = np.resize(bidx, n)
    cx = np.clip(np.round(cb[:, 0] * np.float32(W)), 0, W - 1).astype(np.int64)
    cy = np.clip(np.round(cb[:, 1] * np.float32(H)), 0, H - 1).astype(np.int64)
    caps = np.zeros(NHB, dtype=np.int64)
    for core in range(NCORES):
        cx0 = WS * core - HALO
        sel = (cx >= cx0) & (cx < cx0 + WL)
        cell = (cx[sel] - cx0) * H + cy[sel]
        hb = cell // HB + bidx[sel] * NHB
        cnt = np.bincount(hb, minlength=2 * NHB).reshape(2, NHB).max(axis=0)
        caps = np.maximum(caps, cnt)
    return [int(-(-c // 128)) for c in caps]


def _device_kernel(**inputs):
    _ensure_axon()
    _set_caps(_fit_caps(inputs["currentBlock"], inputs["eventCounts"]))
    ev = _prep_events(inputs["currentBlock"], inputs["eventCounts"])
    if ev is None:
        print("kernel: slab capacity exceeded; numpy fallback")
        return _numpy_forward(inputs)

    from concourse.bass_utils import run_bass_kernel_spmd

    nc = _build_program()
    expts = _expand_tables(inputs["hash_tables"])
    wts = _repack_weights(inputs)

    resc = np.zeros((128, 24), np.float32)
    resc[:, :] = RES.astype(np.float32).reshape(-1)[None, :]
    iota = np.tile(np.arange(SLABMAX, dtype=np.float32)[None, :], (128, 1))

    shared = {"resc": resc, "iotad": iota,
              "km1": wts["km1"], "km2": wts["km2"], "km3": wts["km3"], "km4": wts["km4"],
              "cb1": wts["cb1"], "cb2": wts["cb2"], "cb3": wts["cb3"], "cb4": wts["cb4"],
              "mw0": wts["mw0"], "mw1": wts["mw1"], "mw2": wts["mw2"], "mw3": wts["mw3"],
              "mb0": wts["mb0"], "mb1": wts["mb1"], "mb2": wts["mb2"], "mb3": wts["mb3"]}
    in_maps = []
    for core in range(NCORES):
        m = dict(shared)
        m["coords"] = ev[core]["coords"]
        m["idx16"] = ev[core]["idx16"]
        m["gcnt"] = ev[core]["gcnt"]
        m["expt"] = expts[core]
        m["emask"] = _edge_mask(core)
        in_maps.append(m)

    res = run_bass_kernel_spmd(nc, in_maps, list(range(NCORES)))

    out = np.zeros((B, W, H, T), np.float32)
    for core in range(NCORES):
        o = res.results[core]["out"]           # [20, OUTPX]
        o = o.reshape(T, B, WS, H)
        out[:, WS * core:WS * (core + 1), :, :] = o.transpose(1, 2, 3, 0)
    return out

